# revision 41
# baseline (speedup 1.0000x reference)
"""Trainium2 Bass kernel for nn_L1Wav: 5-level 3D db4 wavelet soft-threshold
denoising of a 256^3 complex volume, SPMD over 8 NeuronCores.

Math notes (verified against the jax reference):
  - The deterministic rng(1000) shift is 0 and the unit-modulus phase cancels
    through the prox, so the computation is exactly:
    5-level 3D DWT -> complex soft-threshold -> inverse DWT.
  - Sharding: volume split along axis 0 (32 planes/core). Levels 1-2 are
    distributed; levels 3-5 replicated. One AllGather of the L1 approx band.

Implementation notes (v3):
  - All separable 2D (q,r) transform passes use a "data-stationary" two-step
    matmul form: step1 out1(r,q') = X(q,r)^T @ W(q,q'), step2
    out2(q',r') = out1(r,q')^T @ W(r,r').  No PE transposes.
  - bf16 matmul datapath (fp32 matmul costs 2 PE instructions per call;
    bf16 costs 1).  PSUM accumulation stays fp32; the soft-threshold factor
    f = 1 - t/|w| is computed in fp32.  End-to-end rel err ~1e-3 vs the
    2e-2 budget.
  - step1/step2 software-pipelined one group deep so the PE's LDWEIGHTS
    (whose lhsT is fresh data, not reusable weights) can hoist.
  - DMAs batched (multi-row stages) and rotated across SP/ACT/POOL queues
    (~0.6us issue cost each).
  - Detail bands stored AD-concatenated per (X,Y) so the inverse a-pass
    loads one contiguous (2h, L*L) tensor per quadrant.
"""
import sys
from contextlib import ExitStack

import numpy as np
import ml_dtypes

sys.path.insert(0, "/opt/trn_rl_repo")

import concourse.bass as bass
import concourse.mybir as mybir
import concourse.tile as tile
from concourse import bacc
from concourse.bass_utils import run_bass_kernel_spmd

DT = mybir.dt.float32
DB = mybir.dt.bfloat16
NPB = ml_dtypes.bfloat16
F = 8
DEC_LO = np.array([-0.010597401784997278, 0.032883011666982945, 0.030841381835986965,
                   -0.18703481171888114, -0.02798376941698385, 0.6308807679295904,
                   0.7148465705525415, 0.23037781330885523])
REC_LO = DEC_LO[::-1].copy()
REC_HI = np.array([((-1) ** n) * DEC_LO[n] for n in range(F)])
DEC_HI = REC_HI[::-1].copy()

NS = [256, 131, 69, 38, 22, 14]     # sizes level 0..5
NCORE = 8
COMPS = ("re", "im")
HALF = {1: 19, 2: 13, 3: 38, 4: 22, 5: 14}      # band rows per a-filter half
INV_OUT_ROWS = {1: 32, 2: 19, 3: 69, 4: 38, 5: 22}
EPS = 1e-30
# forward bc row-group (gb*2L <= 512) and DMA row batch (gb*gbuf rows)
BC_GB = {1: 1, 2: 3, 3: 6, 4: 11, 5: 14}
BC_GBUF = {1: 4, 2: 4, 3: 2, 4: 2, 5: 1}
# inverse bc row-group (gb*P <= 512) and stage rows
IBC_GB = {1: 2, 2: 3, 3: 7, 4: 13, 5: 18}
IBC_RSTAGE = {1: 8, 2: 19, 3: 35, 4: 38, 5: 22}


def W_mat(N, flt):
    L = (N + F - 1) // 2
    W = np.zeros((L, N), dtype=np.float32)
    for l in range(L):
        for j in range(F):
            n = 2 * l + 1 - j
            if 0 <= n < N:
                W[l, n] = flt[j]
    return W


def G_mat(L, crop, flt):
    G = np.zeros((crop, L), dtype=np.float32)
    for t in range(crop):
        for m in range(L):
            j = t + 6 - 2 * m
            if 0 <= j < F:
                G[t, m] = flt[j]
    return G


def host_matrices(core):
    """All weight matrices for one core (lhsT layout: (K, M))."""
    c = core
    m = {}
    for l in range(5):
        W2 = np.concatenate([W_mat(NS[l], DEC_LO), W_mat(NS[l], DEC_HI)], 0)
        m[f"WT{l + 1}"] = np.ascontiguousarray(W2.T)     # (N_{l-1}, 2*N_l)
        glo = G_mat(NS[l + 1], NS[l], REC_LO)
        ghi = G_mat(NS[l + 1], NS[l], REC_HI)
        m[f"IAB{l + 1}"] = np.ascontiguousarray(
            np.concatenate([glo.T, ghi.T], 0))           # (2*N_l, N_{l-1})
    # L1 fwd a-pass (per-core): A1 (38, 44) -> lhsT (44, 38)
    A1 = np.zeros((38, 44), dtype=np.float32)
    slab_lo = 32 * c - 6
    for half, flt in ((0, DEC_LO), (1, DEC_HI)):
        for i in range(19):
            l = 16 * c + i
            for k in range(44):
                n = slab_lo + k
                j = 2 * l + 1 - n
                if 0 <= j < F and 0 <= n < 256:
                    A1[half * 19 + i, k] = flt[j]
    m["A1T"] = np.ascontiguousarray(A1.T)
    # L2 fwd a-pass fused: per-core details (26 rows) + replicated full-lo (69)
    A2 = np.concatenate([W_mat(131, DEC_LO)[8 * c:8 * c + 13],
                         W_mat(131, DEC_HI)[8 * c:8 * c + 13]], 0)   # (26,131)
    A2F = np.concatenate([A2, W_mat(131, DEC_LO)], 0)                # (95,131)
    m["A2FT"] = np.ascontiguousarray(A2F.T)                          # (131,95)
    # L1 inv a-pass: core-independent (38, 32)
    G1a = np.zeros((32, 19), dtype=np.float32)
    G1d = np.zeros((32, 19), dtype=np.float32)
    for u in range(32):
        for v in range(19):
            j = u + 6 - 2 * v
            if 0 <= j < F:
                G1a[u, v] = REC_LO[j]
                G1d[u, v] = REC_HI[j]
    m["IA1"] = np.ascontiguousarray(np.concatenate([G1a.T, G1d.T], 0))
    # L2 inv a-pass (per-core)
    glo1 = G_mat(69, 131, REC_LO)
    ghi1 = G_mat(69, 131, REC_HI)
    g2a_full = glo1[16 * c:16 * c + 19, :]                    # (19, 69)
    g2a13 = glo1[16 * c:16 * c + 19, 8 * c:8 * c + 13]
    g2d13 = ghi1[16 * c:16 * c + 19, 8 * c:8 * c + 13]
    m["IA2"] = np.ascontiguousarray(np.concatenate([g2a13.T, g2d13.T], 0))
    m["IA2LL"] = np.ascontiguousarray(np.concatenate([g2a_full.T, g2d13.T], 0))
    return {k: v.astype(np.float32) for k, v in m.items()}


MAT_SHAPES = {k: v.shape for k, v in host_matrices(0).items()}


def chunks_of(total, size=128):
    return [(i, min(size, total - i)) for i in range(0, total, size)]


class Builder:
    def __init__(self, nc, tc, ctx, thresh):
        self.nc = nc
        self.tc = tc
        self.thresh = float(thresh)
        self.p_dram = ctx.enter_context(
            tc.tile_pool(name="dram", bufs=1, space=bass.MemorySpace.DRAM))
        self.p_wts = ctx.enter_context(tc.tile_pool(name="wts", bufs=1))
        self.p_work = ctx.enter_context(tc.tile_pool(name="work", bufs=1))
        self.p_psum = ctx.enter_context(
            tc.tile_pool(name="psum", bufs=1, space=bass.MemorySpace.PSUM))
        self.mats = {}
        self.dram = {}
        self.uid = 0
        self.dma_ld = 0
        self.dma_st = 0
        self.cp_rr = 0
        self.p1_rr = 0
        self.p2_rr = 0
        self.s1_rr = 0
        self.pool_ok = True

    def _id(self):
        self.uid += 1
        return self.uid

    # ---- helpers -----------------------------------------------------
    def dram_tile(self, name, shape, dtype=DB, addr_space="Local"):
        t = self.p_dram.tile(list(shape), dtype, name=name, tag=name,
                             addr_space=addr_space)
        self.dram[name] = t
        return t

    def sbuf(self, shape, tag, bufs=1, dtype=DB):
        return self.p_work.tile(list(shape), dtype, name=f"t{self._id()}",
                                tag=tag, bufs=bufs)

    def psum(self, shape, tag):
        return self.p_psum.tile(list(shape), DT, name=f"p{self._id()}",
                                tag=tag, bufs=1)

    def load(self, dst, src):
        """Input DMA, rotated across POOL/SP issue queues (ACT is busy with
        copies + threshold).  POOL is skipped while self.pool_ok is False
        (an in-flight collective on the Pool queue would head-of-line block
        loads emitted after it)."""
        self.dma_ld += 1
        if self.pool_ok:
            if self.dma_ld % 2 == 0:
                self.nc.sync.dma_start(dst, src)
            else:
                self.nc.gpsimd.dma_start(dst, src)
        else:
            if self.dma_ld % 2 == 0:
                self.nc.sync.dma_start(dst, src)
            else:
                self.nc.scalar.dma_start(dst, src)

    def store(self, dst, src):
        """Output DMA, rotated across ACT/SP/POOL queues."""
        self.dma_st += 1
        e = self.dma_st % 3
        if e == 0 or not self.pool_ok:
            self.nc.sync.dma_start(dst, src)
        elif e == 1:
            self.nc.scalar.dma_start(dst, src)
        else:
            self.nc.gpsimd.dma_start(dst, src)

    def copy(self, dst, src):
        """PSUM->SBUF copy (casts), alternating ACT/DVE."""
        self.cp_rr += 1
        if self.cp_rr % 2 == 0:
            self.nc.scalar.copy(dst, src)
        else:
            self.nc.vector.tensor_copy(dst, src)

    def load_mat(self, name, dram_ap):
        K, M = dram_ap.shape
        tiles = []
        for (k0, kn) in chunks_of(K):
            t = self.p_wts.tile([kn, M], DB, name=f"{name}_{k0}",
                                tag=f"{name}_{k0}", bufs=1)
            self.nc.sync.dma_start(t[:, :], dram_ap[k0:k0 + kn, :])
            tiles.append((t, k0, kn))
        self.mats[name] = tiles

    def mat_rows(self, name, row0, rown):
        """Pieces of matrix rows [row0, row0+rown) split at SBUF chunk
        boundaries: list of (sbuf_ap_all_cols, local_offset, piece_rows)."""
        out = []
        for (t, k0, kn) in self.mats[name]:
            lo = max(row0, k0)
            hi = min(row0 + rown, k0 + kn)
            if lo < hi:
                out.append((t[lo - k0:hi - k0, :], lo - row0, hi - lo))
        assert sum(p[2] for p in out) == rown, f"{name} rows {row0}+{rown}"
        return out

    # ---- soft-threshold: th = w * (1 - t/|w|), f in fp32 -------------
    def soft_pair(self, s_re, s_im, th_re, th_im, full_shape, sb):
        nc = self.nc
        t = self.thresh
        ta = self.sbuf(full_shape, "TH_A", bufs=2, dtype=DT)[:, :sb, :]
        tb = self.sbuf(full_shape, "TH_B", bufs=2, dtype=DT)[:, :sb, :]
        tc_ = self.sbuf(full_shape, "TH_C", bufs=2, dtype=DT)[:, :sb, :]
        nc.scalar.square(ta, s_re)
        nc.scalar.square(tb, s_im)
        nc.vector.scalar_tensor_tensor(tc_, ta, EPS, tb,
                                       mybir.AluOpType.add,
                                       mybir.AluOpType.add)   # re^2+im^2+eps
        nc.vector.reciprocal_approx_fast(ta, tc_)             # 1/|w|^2
        nc.scalar.sqrt(tb, ta)                                # 1/|w|
        nc.vector.tensor_scalar(tc_, tb, -t, 1.0,
                                mybir.AluOpType.mult,
                                mybir.AluOpType.add)          # f = 1 - t/|w|
        nc.vector.tensor_mul(th_re, s_re, tc_)
        nc.vector.tensor_mul(th_im, s_im, tc_)

    # ---- a-pass (forward levels): dst = lhsT^T @ src ----------------
    def fwd_a(self, lname, src_of, dsts, n, stage=2048):
        """src_of(comp) -> flat (K, n*n) AP, or a list of
        (dst_view_fn, src_ap) loader pairs per lhsT K-chunk where
        dst_view_fn(it) shapes the SBUF destination and src_ap is sliced
        [..., s0:s0+sn] on its last axis.
        dsts: list of (dst_of, out_r0, rn, dst_r0)."""
        nc = self.nc
        lhsT = self.mats[lname]
        M = max(r0 + rn for (_, r0, rn, _) in dsts)
        tot = n * n
        pend = []               # deferred stores (one stage deep)
        for comp in COMPS:
            srcf = src_of(comp)
            multi = isinstance(srcf, list)
            for s0 in range(0, tot, stage):
                sn = min(stage, tot - s0)
                its = []
                for i, (lt, k0, kn) in enumerate(lhsT):
                    it = self.sbuf([kn, stage], f"FA_IN{i}", bufs=2)
                    if multi:
                        for (d0, dn, sap) in srcf[i]:
                            self.load(it[d0:d0 + dn, :sn],
                                      sap[:, s0:s0 + sn])
                    else:
                        self.load(it[:, :sn], srcf[k0:k0 + kn, s0:s0 + sn])
                    its.append(it)
                ot = self.sbuf([M, stage], "FA_OUT", bufs=2)
                for t0 in range(0, sn, 512):
                    tn = min(512, sn - t0)
                    p = self.psum([M, 512], f"PF{(t0 // 512) % 2}")
                    for i, (lt, k0, kn) in enumerate(lhsT):
                        nc.tensor.matmul(p[:, :tn], lt[:, :],
                                         its[i][:, t0:t0 + tn],
                                         start=(i == 0),
                                         stop=(i == len(lhsT) - 1))
                    self.copy(ot[:, t0:t0 + tn], p[:, :tn])
                for (dst_of, r0, rn, d0) in dsts:
                    self.store(dst_of(comp)[d0:d0 + rn, s0:s0 + sn],
                               ot[r0:r0 + rn, :sn])

    # ---- generic forward bc-pass (data-stationary, row-grouped) ------
    def bc_fwd(self, lvl, src_name, g_base, g_count, af, bd, out_cols=None):
        """2D transform of rows [g_base, g_base+g_count) of {src_name}{comp}.
        bd(comp, af, X, Y, g0loc, rows, rx0, h) -> (dest_ap, use_th) | None,
        dest_ap shaped (h, rows, <=L) [already rearranged l b n]."""
        nc = self.nc
        Q = NS[lvl - 1]
        L = NS[lvl]
        cols = out_cols if out_cols is not None else 2 * L
        WT = self.mats[f"WT{lvl}"]
        qch = chunks_of(Q)
        mch = chunks_of(cols)
        gb = BC_GB[lvl]
        G = gb * BC_GBUF[lvl]               # rows staged per DMA batch
        tail = None                          # deferred threshold+stores
        for s0 in range(0, g_count, G):
            sb = min(G, g_count - s0)
            s3 = {}
            pending = []                     # [(s1, s3g, b0, bn), ...]
            # all input loads first, so the previous stage's stores (which
            # wait on its threshold chain) don't head-of-line block them
            ins = {}
            for comp in COMPS:
                src = self.dram[f"{src_name}{comp}"]
                row0 = g_base + s0
                its = []
                for qi, (q0, qn) in enumerate(qch):
                    it = self.sbuf([qn, G, Q], f"IN{qi}", bufs=2)
                    self.load(it[:, :sb, :],
                              src[row0:row0 + sb, q0:q0 + qn, :]
                              .rearrange("b q n -> q b n"))
                    its.append(it)
                ins[comp] = its
            for comp in COMPS:
                its = ins[comp]
                s3g = [self.sbuf([mn, G, cols], f"S3_{mi}_{comp}", bufs=2)
                       for mi, (m0, mn) in enumerate(mch)]
                s3[comp] = s3g
                for b0 in range(0, sb, gb):
                    bn = min(gb, sb - b0)
                    # step1 for this group
                    self.s1_rr += 1
                    s1 = []
                    for ri, (r0, rn) in enumerate(qch):
                        self.p1_rr += 1
                        p1 = self.psum([rn, gb, cols], f"P1{self.p1_rr % 4}")
                        for b in range(bn):
                            for qi, (q0, qn) in enumerate(qch):
                                nc.tensor.matmul(
                                    p1[:, b, :],
                                    its[qi][:, b0 + b, r0:r0 + rn],
                                    WT[qi][0][:, :cols],
                                    start=(qi == 0),
                                    stop=(qi == len(qch) - 1))
                        s = self.sbuf([rn, gb, cols],
                                      f"S1_{ri}_{self.s1_rr % 3}", bufs=2)
                        self.copy(s[:, :bn, :], p1[:, :bn, :])
                        s1.append(s)
                    # delayed step2 (software pipeline, depth 2)
                    pending.append((s1, s3g, b0, bn))
                    if len(pending) > 2:
                        self._bc_step2(*pending.pop(0), gb, qch, mch, WT,
                                       cols)
                if comp == COMPS[-1]:
                    while pending:
                        self._bc_step2(*pending.pop(0), gb, qch, mch, WT,
                                       cols)
            self._bc_tail(s0, sb, s3, lvl, af, bd, mch, L, cols, G)

    def _bc_tail(self, s0, sb, s3, lvl, af, bd, mch, L, cols, G):
        """Threshold + band writes for one staged row batch."""
        for mi, (m0, mn) in enumerate(mch):
            pieces = []
            for X in (0, 1):
                lo = max(m0, X * L)
                hi = min(m0 + mn, (X + 1) * L)
                if lo >= hi:
                    continue
                rr0, h = lo - m0, hi - lo
                rx0 = lo - X * L
                for Y in (0, 1):
                    if Y * L >= cols:
                        continue
                    for comp in COMPS:
                        d = bd(comp, af, X, Y, s0, sb, rx0, h)
                        if d is not None:
                            pieces.append((comp, X, Y, rr0, h, d[0],
                                           d[1]))
            th = {}
            if any(p[6] for p in pieces):
                for comp in COMPS:
                    th[comp] = self.sbuf([mn, G, cols],
                                         f"THO_{mi}_{comp}", bufs=2)
                self.soft_pair(s3["re"][mi][:, :sb, :],
                               s3["im"][mi][:, :sb, :],
                               th["re"][:, :sb, :],
                               th["im"][:, :sb, :],
                               [mn, G, cols], sb)
            for (comp, X, Y, rr0, h, dest, use_th) in pieces:
                st = th[comp] if use_th else s3[comp][mi]
                c1 = min((Y + 1) * L, cols)
                self.store(dest, st[rr0:rr0 + h, :sb, Y * L:c1])

    def _bc_step2(self, s1, s3g, b0, bn, gb, qch, mch, WT, cols):
        nc = self.nc
        for mi, (m0, mn) in enumerate(mch):
            self.p2_rr += 1
            p2 = self.psum([mn, gb, cols], f"P2{self.p2_rr % 2}")
            for b in range(bn):
                for ri, (r0, rn) in enumerate(qch):
                    nc.tensor.matmul(
                        p2[:, b, :],
                        s1[ri][:, b, m0:m0 + mn],
                        WT[ri][0][:, :cols],
                        start=(ri == 0),
                        stop=(ri == len(qch) - 1))
            self.copy(s3g[mi][:, b0:b0 + bn, :], p2[:, :bn, :])

    # ---- inverse a-pass ----------------------------------------------
    def inv_a(self, lvl, band_src, stage=2048, quads=None):
        """band_src(comp, X, Y) -> ([(flat_ap, k0, kn)...], lname).
        Writes OC{lvl}{comp}{Y} flat at X*L*L column offset."""
        nc = self.nc
        L = NS[lvl]
        M = INV_OUT_ROWS[lvl]
        tot = L * L
        if quads is None:
            quads = [(X, Y) for X in (0, 1) for Y in (0, 1)]
        pend = []
        for comp in COMPS:
            for X, Y in quads:
                if True:
                    pieces, lname = band_src(comp, X, Y)
                    lt, _, K = self.mats[lname][0]
                    dst = self.dram[f"OC{lvl}{comp}{Y}"].rearrange(
                        "a l n -> a (l n)")
                    for s0 in range(0, tot, stage):
                        sn = min(stage, tot - s0)
                        it = self.sbuf([K, stage], "FA_IN0", bufs=2)
                        for (ap, k0, kn) in pieces:
                            self.load(it[k0:k0 + kn, :sn],
                                      ap[:, s0:s0 + sn])
                        ot = self.sbuf([M, stage], "FA_OUT", bufs=2)
                        for t0 in range(0, sn, 512):
                            tn = min(512, sn - t0)
                            p = self.psum([M, 512], f"PF{(t0 // 512) % 2}")
                            nc.tensor.matmul(p[:, :tn], lt[:, :],
                                             it[:, t0:t0 + tn],
                                             start=True, stop=True)
                            self.copy(ot[:, t0:t0 + tn], p[:, :tn])
                        self.store(
                            dst[:, X * tot + s0:X * tot + s0 + sn],
                            ot[:, :sn])

    # ---- inverse bc-pass (data-stationary, row-grouped) --------------
    def inv_bc(self, lvl, out_dest):
        """OC{lvl}{comp}{Y} (rows, 2L, L) -> out_dest(comp) (rows, P, P)."""
        nc = self.nc
        rows = INV_OUT_ROWS[lvl]
        L = NS[lvl]
        twoL = 2 * L
        P = NS[lvl - 1]
        iname = f"IAB{lvl}"
        IAB = self.mats[iname]
        lch = chunks_of(twoL)
        nch = chunks_of(L)
        pch = chunks_of(P)
        gb = IBC_GB[lvl]
        rstage = IBC_RSTAGE[lvl]
        sdt = DT if lvl == 1 else DB
        stag = "S2F" if lvl == 1 else "S2B"
        # step2 K pieces: (Y, ni, pk0, pkn, rhs_ap)
        klist = []
        for Y in (0, 1):
            for ni, (n0, nn) in enumerate(nch):
                for (rhs, pk0, pkn) in self.mat_rows(f"IABY{lvl}{Y}", n0, nn):
                    klist.append((Y, ni, pk0, pkn, rhs))
        for comp in COMPS:
            dst = out_dest(comp)
            pending = []
            for s0 in range(0, rows, rstage):
                sb = min(rstage, rows - s0)
                oc = {}
                for Y in (0, 1):
                    src = self.dram[f"OC{lvl}{comp}{Y}"]
                    for li, (l0, ln) in enumerate(lch):
                        t = self.sbuf([ln, rstage, L],
                                      f"IN{2 * li + Y}", bufs=2)
                        self.load(t[:, :sb, :],
                                  src[s0:s0 + sb, l0:l0 + ln, :]
                                  .rearrange("b l n -> l b n"))
                        oc[(Y, li)] = t
                for b0 in range(0, sb, gb):
                    bn = min(gb, sb - b0)
                    self.s1_rr += 1
                    sU = {}
                    for Y in (0, 1):
                        for ni, (n0, nn) in enumerate(nch):
                            self.p1_rr += 1
                            p1 = self.psum([nn, gb, P],
                                           f"P1{self.p1_rr % 4}")
                            for b in range(bn):
                                for li, (l0, ln) in enumerate(lch):
                                    nc.tensor.matmul(
                                        p1[:, b, :],
                                        oc[(Y, li)][:, b0 + b, n0:n0 + nn],
                                        IAB[li][0][:, :],
                                        start=(li == 0),
                                        stop=(li == len(lch) - 1))
                            s = self.sbuf([nn, gb, P],
                                          f"S1_{2 * Y + ni}_{self.s1_rr % 3}",
                                          bufs=2)
                            self.copy(s[:, :bn, :], p1[:, :bn, :])
                            sU[(Y, ni)] = s
                    pending.append((sU, s0 + b0, bn))
                    if len(pending) > 2:
                        self._ibc_step2(*pending.pop(0), gb, pch, klist, P,
                                        dst, sdt, stag)
            while pending:
                self._ibc_step2(*pending.pop(0), gb, pch, klist, P, dst,
                                sdt, stag)

    def _ibc_step2(self, sU, g0, bn, gb, pch, klist, P, dst, sdt, stag):
        nc = self.nc
        for mi, (m0, mn) in enumerate(pch):
            self.p2_rr += 1
            p2 = self.psum([mn, gb, P], f"P2{self.p2_rr % 2}")
            for b in range(bn):
                for k, (Y, ni, pk0, pkn, rhs) in enumerate(klist):
                    nc.tensor.matmul(
                        p2[:, b, :],
                        sU[(Y, ni)][pk0:pk0 + pkn, b, m0:m0 + mn],
                        rhs,
                        start=(k == 0),
                        stop=(k == len(klist) - 1))
            s2 = self.sbuf([mn, gb, P], stag, bufs=3, dtype=sdt)
            self.copy(s2[:, :bn, :], p2[:, :bn, :])
            self.store(dst[g0:g0 + bn, m0:m0 + mn, :]
                       .rearrange("b m n -> m b n"),
                       s2[:, :bn, :])


def build_program(thresh, use_collective=True):
    nc = bacc.Bacc("TRN2", target_bir_lowering=False, debug=False,
                   num_devices=NCORE)
    ext = {}
    for comp in COMPS:
        ext[f"xs_{comp}"] = nc.dram_tensor(f"xs_{comp}", [44, 256, 256], DB,
                                           kind="ExternalInput").ap()
    for name, shp in MAT_SHAPES.items():
        ext[name] = nc.dram_tensor(name, list(shp), DB,
                                   kind="ExternalInput").ap()
    outs = {}
    for comp in COMPS:
        outs[comp] = nc.dram_tensor(f"out_{comp}", [32, 256, 256], DT,
                                    kind="ExternalOutput").ap()

    with tile.TileContext(nc) as tc, ExitStack() as ctx:
        b = Builder(nc, tc, ctx, thresh)

        for name in MAT_SHAPES:
            b.load_mat(name, ext[name])
        for lvl in range(1, 6):
            Ll = NS[lvl]
            for Y in (0, 1):
                b.load_mat(f"IABY{lvl}{Y}",
                           ext[f"IAB{lvl}"][Y * Ll:(Y + 1) * Ll, :])

        for comp in COMPS:
            b.dram[f"xs{comp}"] = ext[f"xs_{comp}"]
            b.dram_tile(f"Af1{comp}", (38, 256, 256))
            b.dram_tile(f"Af2{comp}", (26, 131, 131))
            b.dram_tile(f"Af2F{comp}", (69, 131, 131))
            b.dram_tile(f"Af3{comp}", (76, 69, 69))
            b.dram_tile(f"Af4{comp}", (44, 38, 38))
            b.dram_tile(f"Af5{comp}", (28, 22, 22))
            b.dram_tile(f"VA1full{comp}", (131, 131, 131))
            b.dram_tile(f"VA2full{comp}", (69, 69, 69))
            for lvl, L in ((1, 131), (2, 69), (3, 38), (4, 22), (5, 14)):
                for X in (0, 1):
                    for Y in (0, 1):
                        b.dram_tile(f"B{lvl}{comp}{X}{Y}",
                                    (2 * HALF[lvl], L, L))
            b.dram_tile(f"VA1rec{comp}", (19, 131, 131))
            b.dram_tile(f"VA2rec{comp}", (69, 69, 69))
            b.dram_tile(f"VA3rec{comp}", (38, 38, 38))
            b.dram_tile(f"VA4rec{comp}", (22, 22, 22))
            for lvl, L in ((1, 131), (2, 69), (3, 38), (4, 22), (5, 14)):
                for Y in (0, 1):
                    b.dram_tile(f"OC{lvl}{comp}{Y}",
                                (INV_OUT_ROWS[lvl], 2 * L, L))
        ag1_in = b.dram_tile("ag1_in", (38, 131, 131))
        ag1_out = b.dram_tile("ag1_out", (NCORE * 38, 131, 131),
                              addr_space="Shared")

        def flat(name):
            return lambda comp: b.dram[f"{name}{comp}"].rearrange(
                "a b c -> a (b c)")

        # ============ forward ============
        b.fwd_a("A1T",
                lambda c: ext[f"xs_{c}"].rearrange("a b c -> a (b c)"),
                [(flat("Af1"), 0, 38, 0)], 256)

        def bd1(comp, af, X, Y, g0, rows, rx0, h):
            if af == 0 and X == 0 and Y == 0:
                ci = 0 if comp == "re" else 1
                return (ag1_in[ci * 19 + g0:ci * 19 + g0 + rows,
                               rx0:rx0 + h, :]
                        .rearrange("b l n -> l b n"), False)
            return (b.dram[f"B1{comp}{X}{Y}"]
                    [af * 19 + g0:af * 19 + g0 + rows, rx0:rx0 + h, :]
                    .rearrange("b l n -> l b n"), True)

        # inverse band sources (used both for early off-critical-path
        # quadrants and in the inverse chain)
        def bsrc_rep(lvl, va_rec):
            h0 = HALF[lvl]

            def f(comp, X, Y):
                bt = b.dram[f"B{lvl}{comp}{X}{Y}"].rearrange(
                    "a b c -> a (b c)")
                if X == 0 and Y == 0 and lvl != 5:
                    va = b.dram[f"{va_rec}{comp}"].rearrange(
                        "a b c -> a (b c)")
                    return ([(va, 0, h0), (bt[h0:2 * h0], h0, h0)],
                            f"IAB{lvl}")
                return ([(bt, 0, 2 * h0)], f"IAB{lvl}")
            return f

        def bsrc2(comp, X, Y):
            bt = b.dram[f"B2{comp}{X}{Y}"].rearrange("a b c -> a (b c)")
            if X == 0 and Y == 0:
                va = b.dram[f"VA2rec{comp}"].rearrange("a b c -> a (b c)")
                return ([(va, 0, 69), (bt[13:26], 69, 13)], "IA2LL")
            return ([(bt, 0, 26)], "IA2")

        def bsrc1(comp, X, Y):
            bt = b.dram[f"B1{comp}{X}{Y}"].rearrange("a b c -> a (b c)")
            if X == 0 and Y == 0:
                va = b.dram[f"VA1rec{comp}"].rearrange("a b c -> a (b c)")
                return ([(va, 0, 19), (bt[19:38], 19, 19)], "IA1")
            return ([(bt, 0, 38)], "IA1")

        NON00 = [(0, 1), (1, 0), (1, 1)]

        b.bc_fwd(1, "Af1", 0, 19, 0, bd1)

        if use_collective:
            nc.gpsimd.collective_compute(
                "AllGather", mybir.AluOpType.bypass,
                ins=[ag1_in.opt()], outs=[ag1_out.opt()],
                replica_groups=[list(range(NCORE))])
        else:
            nc.sync.dma_start(ag1_out[0:38], ag1_in[0:38])

        b.pool_ok = False
        b.bc_fwd(1, "Af1", 19, 19, 1, bd1)
        b.pool_ok = True

        # L2 a-pass reads the gathered L1 approx band directly from
        # ag1_out: VA1 row 16k+j lives at ag1_out row 38k + ci*19 + j.
        # Simple 2D row-slices only (dependency tracking vs the collective
        # write is reliable for these).
        agf = ag1_out.rearrange("a b c -> a (b c)")

        def a2f_src(comp):
            ci = 0 if comp == "re" else 1
            c0 = []
            for k in range(7):
                c0.append((16 * k, 16,
                           agf[38 * k + ci * 19:38 * k + ci * 19 + 16]))
            c0.append((112, 16,
                       agf[38 * 7 + ci * 19:38 * 7 + ci * 19 + 16]))
            c1 = [(0, 3, agf[38 * 7 + ci * 19 + 16:38 * 7 + ci * 19 + 19])]
            return [c0, c1]

        b.fwd_a("A2FT", a2f_src,
                [(flat("Af2"), 0, 26, 0), (flat("Af2F"), 26, 69, 0)], 131)

        # L2 lo-lo (replicated, unthresholded) -> VA2full
        def bd_ll(comp, af, X, Y, g0, rows, rx0, h):
            return (b.dram[f"VA2full{comp}"][g0:g0 + rows, rx0:rx0 + h, :]
                    .rearrange("b l n -> l b n"), False)

        b.bc_fwd(2, "Af2F", 0, 69, 0, bd_ll, out_cols=69)

        def bd_rep(lvl):
            def f(comp, af, X, Y, g0, rows, rx0, h):
                h0 = HALF[lvl]
                if af == 0 and X == 0 and Y == 0:
                    if lvl == 2:
                        return None       # aaa2 comes from the bc_ll pass
                    use_th = (lvl == 5)
                    return (b.dram[f"B{lvl}{comp}00"][g0:g0 + rows,
                                                      rx0:rx0 + h, :]
                            .rearrange("b l n -> l b n"), use_th)
                return (b.dram[f"B{lvl}{comp}{X}{Y}"]
                        [af * h0 + g0:af * h0 + g0 + rows, rx0:rx0 + h, :]
                        .rearrange("b l n -> l b n"), True)
            return f

        b.bc_fwd(2, "Af2", 0, 13, 0, bd_rep(2))
        b.bc_fwd(2, "Af2", 13, 13, 1, bd_rep(2))

        b.fwd_a("WT3", flat("VA2full"), [(flat("Af3"), 0, 76, 0)], 69)
        b.bc_fwd(3, "Af3", 0, 38, 0, bd_rep(3))
        b.bc_fwd(3, "Af3", 38, 38, 1, bd_rep(3))
        # off-critical-path: L2 inverse a-pass detail quadrants (only need
        # the forward B2 bands) — fills the small-level trough
        b.inv_a(2, bsrc2, quads=NON00)

        b.fwd_a("WT4",
                lambda c: b.dram[f"B3{c}00"].rearrange("a b c -> a (b c)")
                [0:38], [(flat("Af4"), 0, 44, 0)], 38)
        b.bc_fwd(4, "Af4", 0, 22, 0, bd_rep(4))
        b.bc_fwd(4, "Af4", 22, 22, 1, bd_rep(4))
        # off-critical-path: L1 inverse a-pass detail quadrants
        b.inv_a(1, bsrc1, quads=NON00)

        b.fwd_a("WT5",
                lambda c: b.dram[f"B4{c}00"].rearrange("a b c -> a (b c)")
                [0:22], [(flat("Af5"), 0, 28, 0)], 22)
        b.bc_fwd(5, "Af5", 0, 14, 0, bd_rep(5))
        b.bc_fwd(5, "Af5", 14, 14, 1, bd_rep(5))

        # ============ inverse ============
        b.inv_a(5, bsrc_rep(5, None))
        b.inv_bc(5, lambda c: b.dram[f"VA4rec{c}"])
        b.inv_a(4, bsrc_rep(4, "VA4rec"))
        b.inv_bc(4, lambda c: b.dram[f"VA3rec{c}"])
        b.inv_a(3, bsrc_rep(3, "VA3rec"))
        b.inv_bc(3, lambda c: b.dram[f"VA2rec{c}"])
        b.inv_a(2, bsrc2, quads=[(0, 0)])
        b.inv_bc(2, lambda c: b.dram[f"VA1rec{c}"])
        b.inv_a(1, bsrc1, quads=[(0, 0)])
        b.inv_bc(1, lambda c: outs[c])

    nc.compile()
    return nc


_CACHE = {}


def make_in_maps(x_real, x_imag):
    x_real = np.ascontiguousarray(x_real, dtype=np.float32)
    x_imag = np.ascontiguousarray(x_imag, dtype=np.float32)
    in_maps = []
    for c in range(NCORE):
        m = host_matrices(c)
        slab_lo = 32 * c - 6
        im = {}
        for comp, x in (("re", x_real), ("im", x_imag)):
            s = np.zeros((44, 256, 256), dtype=NPB)
            g0, g1 = max(0, slab_lo), min(256, slab_lo + 44)
            s[g0 - slab_lo:g1 - slab_lo] = x[g0:g1]
            im[f"xs_{comp}"] = s
        for k, v in m.items():
            im[k] = v.astype(NPB)
        in_maps.append(im)
    return in_maps


def kernel(x_real, x_imag, alpha):
    thresh = 1e-3 * float(np.asarray(alpha))
    if thresh not in _CACHE:
        _CACHE[thresh] = build_program(thresh)
    nc = _CACHE[thresh]

    in_maps = make_in_maps(x_real, x_imag)
    res = run_bass_kernel_spmd(nc, in_maps, core_ids=list(range(NCORE)))
    out = np.empty((256, 256, 256), dtype=np.complex64)
    for c in range(NCORE):
        r = res.results[c]
        out[32 * c:32 * c + 32] = r["out_re"] + 1j * r["out_im"]
    return out


# revision 43
# speedup vs baseline: 1.0560x; 1.0560x over previous
"""Trainium2 Bass kernel for nn_L1Wav: 5-level 3D db4 wavelet soft-threshold
denoising of a 256^3 complex volume, SPMD over 8 NeuronCores.

Math notes (verified against the jax reference):
  - The deterministic rng(1000) shift is 0 and the unit-modulus phase cancels
    through the prox, so the computation is exactly:
    5-level 3D DWT -> complex soft-threshold -> inverse DWT.
  - Sharding: volume split along axis 0 (32 planes/core). Levels 1-2 are
    distributed; levels 3-5 replicated. One AllGather of the L1 approx band.

Implementation notes (v3):
  - All separable 2D (q,r) transform passes use a "data-stationary" two-step
    matmul form: step1 out1(r,q') = X(q,r)^T @ W(q,q'), step2
    out2(q',r') = out1(r,q')^T @ W(r,r').  No PE transposes.
  - bf16 matmul datapath (fp32 matmul costs 2 PE instructions per call;
    bf16 costs 1).  PSUM accumulation stays fp32; the soft-threshold factor
    f = 1 - t/|w| is computed in fp32.  End-to-end rel err ~1e-3 vs the
    2e-2 budget.
  - step1/step2 software-pipelined one group deep so the PE's LDWEIGHTS
    (whose lhsT is fresh data, not reusable weights) can hoist.
  - DMAs batched (multi-row stages) and rotated across SP/ACT/POOL queues
    (~0.6us issue cost each).
  - Detail bands stored AD-concatenated per (X,Y) so the inverse a-pass
    loads one contiguous (2h, L*L) tensor per quadrant.
"""
import sys
from contextlib import ExitStack

import numpy as np
import ml_dtypes

sys.path.insert(0, "/opt/trn_rl_repo")

import concourse.bass as bass
import concourse.mybir as mybir
import concourse.tile as tile
from concourse import bacc
from concourse.bass_utils import run_bass_kernel_spmd

DT = mybir.dt.float32
DB = mybir.dt.bfloat16
NPB = ml_dtypes.bfloat16
F = 8
DEC_LO = np.array([-0.010597401784997278, 0.032883011666982945, 0.030841381835986965,
                   -0.18703481171888114, -0.02798376941698385, 0.6308807679295904,
                   0.7148465705525415, 0.23037781330885523])
REC_LO = DEC_LO[::-1].copy()
REC_HI = np.array([((-1) ** n) * DEC_LO[n] for n in range(F)])
DEC_HI = REC_HI[::-1].copy()

NS = [256, 131, 69, 38, 22, 14]     # sizes level 0..5
NCORE = 8
COMPS = ("re", "im")
HALF = {1: 19, 2: 13, 3: 38, 4: 22, 5: 14}      # band rows per a-filter half
INV_OUT_ROWS = {1: 32, 2: 19, 3: 69, 4: 38, 5: 22}
EPS = 1e-30
# forward bc row-group (gb*2L <= 512) and DMA row batch (gb*gbuf rows)
BC_GB = {1: 1, 2: 3, 3: 6, 4: 11, 5: 14}
BC_GBUF = {1: 2, 2: 2, 3: 1, 4: 1, 5: 1}
# inverse bc row-group (gb*P <= 512) and stage rows
IBC_GB = {1: 2, 2: 3, 3: 7, 4: 13, 5: 18}
IBC_RSTAGE = {1: 8, 2: 10, 3: 23, 4: 38, 5: 22}


def W_mat(N, flt):
    L = (N + F - 1) // 2
    W = np.zeros((L, N), dtype=np.float32)
    for l in range(L):
        for j in range(F):
            n = 2 * l + 1 - j
            if 0 <= n < N:
                W[l, n] = flt[j]
    return W


def G_mat(L, crop, flt):
    G = np.zeros((crop, L), dtype=np.float32)
    for t in range(crop):
        for m in range(L):
            j = t + 6 - 2 * m
            if 0 <= j < F:
                G[t, m] = flt[j]
    return G


def host_matrices(core):
    """All weight matrices for one core (lhsT layout: (K, M))."""
    c = core
    m = {}
    for l in range(5):
        W2 = np.concatenate([W_mat(NS[l], DEC_LO), W_mat(NS[l], DEC_HI)], 0)
        m[f"WT{l + 1}"] = np.ascontiguousarray(W2.T)     # (N_{l-1}, 2*N_l)
        glo = G_mat(NS[l + 1], NS[l], REC_LO)
        ghi = G_mat(NS[l + 1], NS[l], REC_HI)
        m[f"IAB{l + 1}"] = np.ascontiguousarray(
            np.concatenate([glo.T, ghi.T], 0))           # (2*N_l, N_{l-1})
    # L1 fwd a-pass (per-core): A1 (38, 44) -> lhsT (44, 38)
    A1 = np.zeros((38, 44), dtype=np.float32)
    slab_lo = 32 * c - 6
    for half, flt in ((0, DEC_LO), (1, DEC_HI)):
        for i in range(19):
            l = 16 * c + i
            for k in range(44):
                n = slab_lo + k
                j = 2 * l + 1 - n
                if 0 <= j < F and 0 <= n < 256:
                    A1[half * 19 + i, k] = flt[j]
    m["A1T"] = np.ascontiguousarray(A1.T)
    # L2 fwd a-pass fused: per-core details (26 rows) + replicated full-lo (69)
    A2 = np.concatenate([W_mat(131, DEC_LO)[8 * c:8 * c + 13],
                         W_mat(131, DEC_HI)[8 * c:8 * c + 13]], 0)   # (26,131)
    A2F = np.concatenate([A2, W_mat(131, DEC_LO)], 0)                # (95,131)
    m["A2FT"] = np.ascontiguousarray(A2F.T)                          # (131,95)
    # L1 inv a-pass: core-independent (38, 32)
    G1a = np.zeros((32, 19), dtype=np.float32)
    G1d = np.zeros((32, 19), dtype=np.float32)
    for u in range(32):
        for v in range(19):
            j = u + 6 - 2 * v
            if 0 <= j < F:
                G1a[u, v] = REC_LO[j]
                G1d[u, v] = REC_HI[j]
    m["IA1"] = np.ascontiguousarray(np.concatenate([G1a.T, G1d.T], 0))
    # L2 inv a-pass (per-core)
    glo1 = G_mat(69, 131, REC_LO)
    ghi1 = G_mat(69, 131, REC_HI)
    g2a_full = glo1[16 * c:16 * c + 19, :]                    # (19, 69)
    g2a13 = glo1[16 * c:16 * c + 19, 8 * c:8 * c + 13]
    g2d13 = ghi1[16 * c:16 * c + 19, 8 * c:8 * c + 13]
    m["IA2"] = np.ascontiguousarray(np.concatenate([g2a13.T, g2d13.T], 0))
    m["IA2LL"] = np.ascontiguousarray(np.concatenate([g2a_full.T, g2d13.T], 0))
    return {k: v.astype(np.float32) for k, v in m.items()}


MAT_SHAPES = {k: v.shape for k, v in host_matrices(0).items()}


def chunks_of(total, size=128):
    return [(i, min(size, total - i)) for i in range(0, total, size)]


class Builder:
    def __init__(self, nc, tc, ctx, thresh):
        self.nc = nc
        self.tc = tc
        self.thresh = float(thresh)
        self.p_dram = ctx.enter_context(
            tc.tile_pool(name="dram", bufs=1, space=bass.MemorySpace.DRAM))
        self.p_wts = ctx.enter_context(tc.tile_pool(name="wts", bufs=1))
        self.p_work = ctx.enter_context(tc.tile_pool(name="work", bufs=1))
        self.p_psum = ctx.enter_context(
            tc.tile_pool(name="psum", bufs=1, space=bass.MemorySpace.PSUM))
        self.mats = {}
        self.dram = {}
        self.uid = 0
        self.dma_ld = 0
        self.dma_st = 0
        self.cp_rr = 0
        self.p1_rr = 0
        self.p2_rr = 0
        self.s1_rr = 0
        self.pool_ok = True

    def _id(self):
        self.uid += 1
        return self.uid

    # ---- helpers -----------------------------------------------------
    def dram_tile(self, name, shape, dtype=DB, addr_space="Local"):
        t = self.p_dram.tile(list(shape), dtype, name=name, tag=name,
                             addr_space=addr_space)
        self.dram[name] = t
        return t

    def sbuf(self, shape, tag, bufs=1, dtype=DB):
        return self.p_work.tile(list(shape), dtype, name=f"t{self._id()}",
                                tag=tag, bufs=bufs)

    def psum(self, shape, tag):
        return self.p_psum.tile(list(shape), DT, name=f"p{self._id()}",
                                tag=tag, bufs=1)

    def load(self, dst, src):
        """Input DMA, rotated across POOL/SP issue queues (ACT is busy with
        copies + threshold).  POOL is skipped while self.pool_ok is False
        (an in-flight collective on the Pool queue would head-of-line block
        loads emitted after it)."""
        self.dma_ld += 1
        if self.pool_ok:
            if self.dma_ld % 2 == 0:
                self.nc.sync.dma_start(dst, src)
            else:
                self.nc.gpsimd.dma_start(dst, src)
        else:
            if self.dma_ld % 2 == 0:
                self.nc.sync.dma_start(dst, src)
            else:
                self.nc.scalar.dma_start(dst, src)

    def store(self, dst, src):
        """Output DMA, rotated across ACT/SP/POOL queues."""
        self.dma_st += 1
        e = self.dma_st % 3
        if e == 0 or not self.pool_ok:
            self.nc.sync.dma_start(dst, src)
        elif e == 1:
            self.nc.scalar.dma_start(dst, src)
        else:
            self.nc.gpsimd.dma_start(dst, src)

    def copy(self, dst, src):
        """PSUM->SBUF copy (casts), alternating ACT/DVE."""
        self.cp_rr += 1
        if self.cp_rr % 2 == 0:
            self.nc.scalar.copy(dst, src)
        else:
            self.nc.vector.tensor_copy(dst, src)

    def load_mat(self, name, dram_ap):
        K, M = dram_ap.shape
        tiles = []
        for (k0, kn) in chunks_of(K):
            t = self.p_wts.tile([kn, M], DB, name=f"{name}_{k0}",
                                tag=f"{name}_{k0}", bufs=1)
            self.nc.sync.dma_start(t[:, :], dram_ap[k0:k0 + kn, :])
            tiles.append((t, k0, kn))
        self.mats[name] = tiles

    def mat_rows(self, name, row0, rown):
        """Pieces of matrix rows [row0, row0+rown) split at SBUF chunk
        boundaries: list of (sbuf_ap_all_cols, local_offset, piece_rows)."""
        out = []
        for (t, k0, kn) in self.mats[name]:
            lo = max(row0, k0)
            hi = min(row0 + rown, k0 + kn)
            if lo < hi:
                out.append((t[lo - k0:hi - k0, :], lo - row0, hi - lo))
        assert sum(p[2] for p in out) == rown, f"{name} rows {row0}+{rown}"
        return out

    # ---- soft-threshold: th = w * (1 - t/|w|), f in fp32 -------------
    def soft_pair(self, s_re, s_im, th_re, th_im, full_shape, sb):
        nc = self.nc
        t = self.thresh
        ta = self.sbuf(full_shape, "TH_A", bufs=2, dtype=DT)[:, :sb, :]
        tb = self.sbuf(full_shape, "TH_B", bufs=2, dtype=DT)[:, :sb, :]
        tc_ = self.sbuf(full_shape, "TH_C", bufs=2, dtype=DT)[:, :sb, :]
        nc.scalar.square(ta, s_re)
        nc.scalar.square(tb, s_im)
        nc.vector.scalar_tensor_tensor(tc_, ta, EPS, tb,
                                       mybir.AluOpType.add,
                                       mybir.AluOpType.add)   # re^2+im^2+eps
        nc.vector.reciprocal_approx_fast(ta, tc_)             # 1/|w|^2
        nc.scalar.sqrt(tb, ta)                                # 1/|w|
        nc.vector.tensor_scalar(tc_, tb, -t, 1.0,
                                mybir.AluOpType.mult,
                                mybir.AluOpType.add)          # f = 1 - t/|w|
        nc.vector.tensor_mul(th_re, s_re, tc_)
        nc.vector.tensor_mul(th_im, s_im, tc_)

    # ---- a-pass (forward levels): dst = lhsT^T @ src ----------------
    def fwd_a(self, lname, src_of, dsts, n, stage=2048):
        """src_of(comp) -> flat (K, n*n) AP, or a list of
        (dst_view_fn, src_ap) loader pairs per lhsT K-chunk where
        dst_view_fn(it) shapes the SBUF destination and src_ap is sliced
        [..., s0:s0+sn] on its last axis.
        dsts: list of (dst_of, out_r0, rn, dst_r0)."""
        nc = self.nc
        lhsT = self.mats[lname]
        M = max(r0 + rn for (_, r0, rn, _) in dsts)
        tot = n * n
        pend = []               # deferred stores (one stage deep)
        for comp in COMPS:
            srcf = src_of(comp)
            multi = isinstance(srcf, list)
            for s0 in range(0, tot, stage):
                sn = min(stage, tot - s0)
                its = []
                for i, (lt, k0, kn) in enumerate(lhsT):
                    it = self.sbuf([kn, stage], f"FA_IN{i}", bufs=2)
                    if multi:
                        for (d0, dn, sap) in srcf[i]:
                            self.load(it[d0:d0 + dn, :sn],
                                      sap[:, s0:s0 + sn])
                    else:
                        self.load(it[:, :sn], srcf[k0:k0 + kn, s0:s0 + sn])
                    its.append(it)
                ot = self.sbuf([M, stage], "FA_OUT", bufs=2)
                for t0 in range(0, sn, 512):
                    tn = min(512, sn - t0)
                    p = self.psum([M, 512], f"PF{(t0 // 512) % 2}")
                    for i, (lt, k0, kn) in enumerate(lhsT):
                        nc.tensor.matmul(p[:, :tn], lt[:, :],
                                         its[i][:, t0:t0 + tn],
                                         start=(i == 0),
                                         stop=(i == len(lhsT) - 1))
                    self.copy(ot[:, t0:t0 + tn], p[:, :tn])
                for (dst_of, r0, rn, d0) in dsts:
                    self.store(dst_of(comp)[d0:d0 + rn, s0:s0 + sn],
                               ot[r0:r0 + rn, :sn])

    # ---- generic forward bc-pass (data-stationary, row-grouped) ------
    def bc_fwd(self, lvl, src_name, g_base, g_count, af, bd, out_cols=None):
        """2D transform of rows [g_base, g_base+g_count) of {src_name}{comp}.
        bd(comp, af, X, Y, g0loc, rows, rx0, h) -> (dest_ap, use_th) | None,
        dest_ap shaped (h, rows, <=L) [already rearranged l b n]."""
        nc = self.nc
        Q = NS[lvl - 1]
        L = NS[lvl]
        cols = out_cols if out_cols is not None else 2 * L
        WT = self.mats[f"WT{lvl}"]
        qch = chunks_of(Q)
        mch = chunks_of(cols)
        gb = BC_GB[lvl]
        G = gb * BC_GBUF[lvl]               # rows staged per DMA batch
        tail = None                          # deferred threshold+stores
        for s0 in range(0, g_count, G):
            sb = min(G, g_count - s0)
            s3 = {}
            pending = []                     # [(s1, s3g, b0, bn), ...]
            # all input loads first, so the previous stage's stores (which
            # wait on its threshold chain) don't head-of-line block them
            ins = {}
            for comp in COMPS:
                src = self.dram[f"{src_name}{comp}"]
                row0 = g_base + s0
                its = []
                for qi, (q0, qn) in enumerate(qch):
                    it = self.sbuf([qn, G, Q], f"IN{qi}", bufs=2)
                    self.load(it[:, :sb, :],
                              src[row0:row0 + sb, q0:q0 + qn, :]
                              .rearrange("b q n -> q b n"))
                    its.append(it)
                ins[comp] = its
            for comp in COMPS:
                its = ins[comp]
                s3g = [self.sbuf([mn, G, cols], f"S3_{mi}_{comp}", bufs=2)
                       for mi, (m0, mn) in enumerate(mch)]
                s3[comp] = s3g
                for b0 in range(0, sb, gb):
                    bn = min(gb, sb - b0)
                    # step1 for this group
                    self.s1_rr += 1
                    s1 = []
                    for ri, (r0, rn) in enumerate(qch):
                        self.p1_rr += 1
                        p1 = self.psum([rn, gb, cols], f"P1{self.p1_rr % 4}")
                        for b in range(bn):
                            for qi, (q0, qn) in enumerate(qch):
                                nc.tensor.matmul(
                                    p1[:, b, :],
                                    its[qi][:, b0 + b, r0:r0 + rn],
                                    WT[qi][0][:, :cols],
                                    start=(qi == 0),
                                    stop=(qi == len(qch) - 1))
                        s = self.sbuf([rn, gb, cols],
                                      f"S1_{ri}_{self.s1_rr % 3}", bufs=2)
                        self.copy(s[:, :bn, :], p1[:, :bn, :])
                        s1.append(s)
                    # delayed step2 (software pipeline, depth 2)
                    pending.append((s1, s3g, b0, bn))
                    if len(pending) > 2:
                        self._bc_step2(*pending.pop(0), gb, qch, mch, WT,
                                       cols)
                if comp == COMPS[-1]:
                    while pending:
                        self._bc_step2(*pending.pop(0), gb, qch, mch, WT,
                                       cols)
            self._bc_tail(s0, sb, s3, lvl, af, bd, mch, L, cols, G)

    def _bc_tail(self, s0, sb, s3, lvl, af, bd, mch, L, cols, G):
        """Threshold + band writes for one staged row batch."""
        for mi, (m0, mn) in enumerate(mch):
            pieces = []
            for X in (0, 1):
                lo = max(m0, X * L)
                hi = min(m0 + mn, (X + 1) * L)
                if lo >= hi:
                    continue
                rr0, h = lo - m0, hi - lo
                rx0 = lo - X * L
                for Y in (0, 1):
                    if Y * L >= cols:
                        continue
                    for comp in COMPS:
                        d = bd(comp, af, X, Y, s0, sb, rx0, h)
                        if d is not None:
                            pieces.append((comp, X, Y, rr0, h, d[0],
                                           d[1]))
            th = {}
            if any(p[6] for p in pieces):
                for comp in COMPS:
                    th[comp] = self.sbuf([mn, G, cols],
                                         f"THO_{mi}_{comp}", bufs=2)
                self.soft_pair(s3["re"][mi][:, :sb, :],
                               s3["im"][mi][:, :sb, :],
                               th["re"][:, :sb, :],
                               th["im"][:, :sb, :],
                               [mn, G, cols], sb)
            for (comp, X, Y, rr0, h, dest, use_th) in pieces:
                st = th[comp] if use_th else s3[comp][mi]
                c1 = min((Y + 1) * L, cols)
                self.store(dest, st[rr0:rr0 + h, :sb, Y * L:c1])

    def _bc_step2(self, s1, s3g, b0, bn, gb, qch, mch, WT, cols):
        nc = self.nc
        for mi, (m0, mn) in enumerate(mch):
            self.p2_rr += 1
            p2 = self.psum([mn, gb, cols], f"P2{self.p2_rr % 2}")
            for b in range(bn):
                for ri, (r0, rn) in enumerate(qch):
                    nc.tensor.matmul(
                        p2[:, b, :],
                        s1[ri][:, b, m0:m0 + mn],
                        WT[ri][0][:, :cols],
                        start=(ri == 0),
                        stop=(ri == len(qch) - 1))
            self.copy(s3g[mi][:, b0:b0 + bn, :], p2[:, :bn, :])

    # ---- inverse a-pass ----------------------------------------------
    def inv_a(self, lvl, band_src, stage=2048, quads=None):
        """band_src(comp, X, Y) -> ([(flat_ap, k0, kn)...], lname).
        Writes OC{lvl}{comp}{Y} flat at X*L*L column offset."""
        nc = self.nc
        L = NS[lvl]
        M = INV_OUT_ROWS[lvl]
        tot = L * L
        if quads is None:
            quads = [(X, Y) for X in (0, 1) for Y in (0, 1)]
        pend = []
        for comp in COMPS:
            for X, Y in quads:
                if True:
                    pieces, lname = band_src(comp, X, Y)
                    lt, _, K = self.mats[lname][0]
                    dst = self.dram[f"OC{lvl}{comp}{Y}"].rearrange(
                        "a l n -> a (l n)")
                    for s0 in range(0, tot, stage):
                        sn = min(stage, tot - s0)
                        it = self.sbuf([K, stage], "FA_IN0", bufs=2)
                        for (ap, k0, kn) in pieces:
                            self.load(it[k0:k0 + kn, :sn],
                                      ap[:, s0:s0 + sn])
                        ot = self.sbuf([M, stage], "FA_OUT", bufs=2)
                        for t0 in range(0, sn, 512):
                            tn = min(512, sn - t0)
                            p = self.psum([M, 512], f"PF{(t0 // 512) % 2}")
                            nc.tensor.matmul(p[:, :tn], lt[:, :],
                                             it[:, t0:t0 + tn],
                                             start=True, stop=True)
                            self.copy(ot[:, t0:t0 + tn], p[:, :tn])
                        self.store(
                            dst[:, X * tot + s0:X * tot + s0 + sn],
                            ot[:, :sn])

    # ---- inverse bc-pass (data-stationary, row-grouped) --------------
    def inv_bc(self, lvl, out_dest):
        """OC{lvl}{comp}{Y} (rows, 2L, L) -> out_dest(comp) (rows, P, P)."""
        nc = self.nc
        rows = INV_OUT_ROWS[lvl]
        L = NS[lvl]
        twoL = 2 * L
        P = NS[lvl - 1]
        iname = f"IAB{lvl}"
        IAB = self.mats[iname]
        lch = chunks_of(twoL)
        nch = chunks_of(L)
        pch = chunks_of(P)
        gb = IBC_GB[lvl]
        rstage = IBC_RSTAGE[lvl]
        sdt = DT if lvl == 1 else DB
        stag = "S2F" if lvl == 1 else "S2B"
        # step2 K pieces: (Y, ni, pk0, pkn, rhs_ap)
        klist = []
        for Y in (0, 1):
            for ni, (n0, nn) in enumerate(nch):
                for (rhs, pk0, pkn) in self.mat_rows(f"IABY{lvl}{Y}", n0, nn):
                    klist.append((Y, ni, pk0, pkn, rhs))
        for comp in COMPS:
            dst = out_dest(comp)
            pending = []
            for s0 in range(0, rows, rstage):
                sb = min(rstage, rows - s0)
                oc = {}
                for Y in (0, 1):
                    src = self.dram[f"OC{lvl}{comp}{Y}"]
                    for li, (l0, ln) in enumerate(lch):
                        t = self.sbuf([ln, rstage, L],
                                      f"IN{2 * li + Y}", bufs=2)
                        self.load(t[:, :sb, :],
                                  src[s0:s0 + sb, l0:l0 + ln, :]
                                  .rearrange("b l n -> l b n"))
                        oc[(Y, li)] = t
                for b0 in range(0, sb, gb):
                    bn = min(gb, sb - b0)
                    self.s1_rr += 1
                    sU = {}
                    for Y in (0, 1):
                        for ni, (n0, nn) in enumerate(nch):
                            self.p1_rr += 1
                            p1 = self.psum([nn, gb, P],
                                           f"P1{self.p1_rr % 4}")
                            for b in range(bn):
                                for li, (l0, ln) in enumerate(lch):
                                    nc.tensor.matmul(
                                        p1[:, b, :],
                                        oc[(Y, li)][:, b0 + b, n0:n0 + nn],
                                        IAB[li][0][:, :],
                                        start=(li == 0),
                                        stop=(li == len(lch) - 1))
                            s = self.sbuf([nn, gb, P],
                                          f"S1_{2 * Y + ni}_{self.s1_rr % 3}",
                                          bufs=2)
                            self.copy(s[:, :bn, :], p1[:, :bn, :])
                            sU[(Y, ni)] = s
                    pending.append((sU, s0 + b0, bn))
                    if len(pending) > 2:
                        self._ibc_step2(*pending.pop(0), gb, pch, klist, P,
                                        dst, sdt, stag)
            while pending:
                self._ibc_step2(*pending.pop(0), gb, pch, klist, P, dst,
                                sdt, stag)

    def _ibc_step2(self, sU, g0, bn, gb, pch, klist, P, dst, sdt, stag):
        nc = self.nc
        for mi, (m0, mn) in enumerate(pch):
            self.p2_rr += 1
            p2 = self.psum([mn, gb, P], f"P2{self.p2_rr % 2}")
            for b in range(bn):
                for k, (Y, ni, pk0, pkn, rhs) in enumerate(klist):
                    nc.tensor.matmul(
                        p2[:, b, :],
                        sU[(Y, ni)][pk0:pk0 + pkn, b, m0:m0 + mn],
                        rhs,
                        start=(k == 0),
                        stop=(k == len(klist) - 1))
            s2 = self.sbuf([mn, gb, P], stag, bufs=3, dtype=sdt)
            self.copy(s2[:, :bn, :], p2[:, :bn, :])
            self.store(dst[g0:g0 + bn, m0:m0 + mn, :]
                       .rearrange("b m n -> m b n"),
                       s2[:, :bn, :])


def build_program(thresh, use_collective=True):
    nc = bacc.Bacc("TRN2", target_bir_lowering=False, debug=False,
                   num_devices=NCORE)
    ext = {}
    for comp in COMPS:
        ext[f"xs_{comp}"] = nc.dram_tensor(f"xs_{comp}", [44, 256, 256], DB,
                                           kind="ExternalInput").ap()
    for name, shp in MAT_SHAPES.items():
        ext[name] = nc.dram_tensor(name, list(shp), DB,
                                   kind="ExternalInput").ap()
    outs = {}
    for comp in COMPS:
        outs[comp] = nc.dram_tensor(f"out_{comp}", [32, 256, 256], DT,
                                    kind="ExternalOutput").ap()

    with tile.TileContext(nc) as tc, ExitStack() as ctx:
        b = Builder(nc, tc, ctx, thresh)

        for name in MAT_SHAPES:
            b.load_mat(name, ext[name])
        for lvl in range(1, 6):
            Ll = NS[lvl]
            for Y in (0, 1):
                b.load_mat(f"IABY{lvl}{Y}",
                           ext[f"IAB{lvl}"][Y * Ll:(Y + 1) * Ll, :])

        for comp in COMPS:
            b.dram[f"xs{comp}"] = ext[f"xs_{comp}"]
            b.dram_tile(f"Af1{comp}", (38, 256, 256))
            b.dram_tile(f"Af2{comp}", (26, 131, 131))
            b.dram_tile(f"Af2F{comp}", (69, 131, 131))
            b.dram_tile(f"Af3{comp}", (76, 69, 69))
            b.dram_tile(f"Af4{comp}", (44, 38, 38))
            b.dram_tile(f"Af5{comp}", (28, 22, 22))
            b.dram_tile(f"VA1full{comp}", (131, 131, 131))
            b.dram_tile(f"VA2full{comp}", (69, 69, 69))
            for lvl, L in ((1, 131), (2, 69), (3, 38), (4, 22), (5, 14)):
                for X in (0, 1):
                    for Y in (0, 1):
                        b.dram_tile(f"B{lvl}{comp}{X}{Y}",
                                    (2 * HALF[lvl], L, L))
            b.dram_tile(f"VA1rec{comp}", (19, 131, 131))
            b.dram_tile(f"VA2rec{comp}", (69, 69, 69))
            b.dram_tile(f"VA3rec{comp}", (38, 38, 38))
            b.dram_tile(f"VA4rec{comp}", (22, 22, 22))
            for lvl, L in ((1, 131), (2, 69), (3, 38), (4, 22), (5, 14)):
                for Y in (0, 1):
                    b.dram_tile(f"OC{lvl}{comp}{Y}",
                                (INV_OUT_ROWS[lvl], 2 * L, L))
        ag1_in = b.dram_tile("ag1_in", (38, 131, 131))
        ag1_out = b.dram_tile("ag1_out", (NCORE * 38, 131, 131),
                              addr_space="Shared")

        def flat(name):
            return lambda comp: b.dram[f"{name}{comp}"].rearrange(
                "a b c -> a (b c)")

        # ============ forward ============
        b.fwd_a("A1T",
                lambda c: ext[f"xs_{c}"].rearrange("a b c -> a (b c)"),
                [(flat("Af1"), 0, 38, 0)], 256)

        def bd1(comp, af, X, Y, g0, rows, rx0, h):
            if af == 0 and X == 0 and Y == 0:
                ci = 0 if comp == "re" else 1
                return (ag1_in[ci * 19 + g0:ci * 19 + g0 + rows,
                               rx0:rx0 + h, :]
                        .rearrange("b l n -> l b n"), False)
            return (b.dram[f"B1{comp}{X}{Y}"]
                    [af * 19 + g0:af * 19 + g0 + rows, rx0:rx0 + h, :]
                    .rearrange("b l n -> l b n"), True)

        # inverse band sources (used both for early off-critical-path
        # quadrants and in the inverse chain)
        def bsrc_rep(lvl, va_rec):
            h0 = HALF[lvl]

            def f(comp, X, Y):
                bt = b.dram[f"B{lvl}{comp}{X}{Y}"].rearrange(
                    "a b c -> a (b c)")
                if X == 0 and Y == 0 and lvl != 5:
                    va = b.dram[f"{va_rec}{comp}"].rearrange(
                        "a b c -> a (b c)")
                    return ([(va, 0, h0), (bt[h0:2 * h0], h0, h0)],
                            f"IAB{lvl}")
                return ([(bt, 0, 2 * h0)], f"IAB{lvl}")
            return f

        def bsrc2(comp, X, Y):
            bt = b.dram[f"B2{comp}{X}{Y}"].rearrange("a b c -> a (b c)")
            if X == 0 and Y == 0:
                va = b.dram[f"VA2rec{comp}"].rearrange("a b c -> a (b c)")
                return ([(va, 0, 69), (bt[13:26], 69, 13)], "IA2LL")
            return ([(bt, 0, 26)], "IA2")

        def bsrc1(comp, X, Y):
            bt = b.dram[f"B1{comp}{X}{Y}"].rearrange("a b c -> a (b c)")
            if X == 0 and Y == 0:
                va = b.dram[f"VA1rec{comp}"].rearrange("a b c -> a (b c)")
                return ([(va, 0, 19), (bt[19:38], 19, 19)], "IA1")
            return ([(bt, 0, 38)], "IA1")

        NON00 = [(0, 1), (1, 0), (1, 1)]

        b.bc_fwd(1, "Af1", 0, 19, 0, bd1)

        if use_collective:
            nc.gpsimd.collective_compute(
                "AllGather", mybir.AluOpType.bypass,
                ins=[ag1_in.opt()], outs=[ag1_out.opt()],
                replica_groups=[list(range(NCORE))])
        else:
            nc.sync.dma_start(ag1_out[0:38], ag1_in[0:38])

        b.pool_ok = False
        b.bc_fwd(1, "Af1", 19, 19, 1, bd1)
        b.pool_ok = True

        # L2 a-pass reads the gathered L1 approx band directly from
        # ag1_out: VA1 row 16k+j lives at ag1_out row 38k + ci*19 + j.
        # Simple 2D row-slices only (dependency tracking vs the collective
        # write is reliable for these).
        agf = ag1_out.rearrange("a b c -> a (b c)")

        def a2f_src(comp):
            ci = 0 if comp == "re" else 1
            c0 = []
            for k in range(7):
                c0.append((16 * k, 16,
                           agf[38 * k + ci * 19:38 * k + ci * 19 + 16]))
            c0.append((112, 16,
                       agf[38 * 7 + ci * 19:38 * 7 + ci * 19 + 16]))
            c1 = [(0, 3, agf[38 * 7 + ci * 19 + 16:38 * 7 + ci * 19 + 19])]
            return [c0, c1]

        b.fwd_a("A2FT", a2f_src,
                [(flat("Af2"), 0, 26, 0), (flat("Af2F"), 26, 69, 0)], 131)

        # L2 lo-lo (replicated, unthresholded) -> VA2full
        def bd_ll(comp, af, X, Y, g0, rows, rx0, h):
            return (b.dram[f"VA2full{comp}"][g0:g0 + rows, rx0:rx0 + h, :]
                    .rearrange("b l n -> l b n"), False)

        b.bc_fwd(2, "Af2F", 0, 69, 0, bd_ll, out_cols=69)

        def bd_rep(lvl):
            def f(comp, af, X, Y, g0, rows, rx0, h):
                h0 = HALF[lvl]
                if af == 0 and X == 0 and Y == 0:
                    if lvl == 2:
                        return None       # aaa2 comes from the bc_ll pass
                    use_th = (lvl == 5)
                    return (b.dram[f"B{lvl}{comp}00"][g0:g0 + rows,
                                                      rx0:rx0 + h, :]
                            .rearrange("b l n -> l b n"), use_th)
                return (b.dram[f"B{lvl}{comp}{X}{Y}"]
                        [af * h0 + g0:af * h0 + g0 + rows, rx0:rx0 + h, :]
                        .rearrange("b l n -> l b n"), True)
            return f

        b.bc_fwd(2, "Af2", 0, 13, 0, bd_rep(2))
        b.bc_fwd(2, "Af2", 13, 13, 1, bd_rep(2))

        b.fwd_a("WT3", flat("VA2full"), [(flat("Af3"), 0, 76, 0)], 69)
        b.bc_fwd(3, "Af3", 0, 38, 0, bd_rep(3))
        b.bc_fwd(3, "Af3", 38, 38, 1, bd_rep(3))
        # off-critical-path: L2 inverse a-pass detail quadrants (only need
        # the forward B2 bands) — fills the small-level trough
        b.inv_a(2, bsrc2, quads=NON00)

        b.fwd_a("WT4",
                lambda c: b.dram[f"B3{c}00"].rearrange("a b c -> a (b c)")
                [0:38], [(flat("Af4"), 0, 44, 0)], 38)
        b.bc_fwd(4, "Af4", 0, 22, 0, bd_rep(4))
        b.bc_fwd(4, "Af4", 22, 22, 1, bd_rep(4))
        # off-critical-path: L1 inverse a-pass detail quadrants
        b.inv_a(1, bsrc1, quads=NON00)

        b.fwd_a("WT5",
                lambda c: b.dram[f"B4{c}00"].rearrange("a b c -> a (b c)")
                [0:22], [(flat("Af5"), 0, 28, 0)], 22)
        b.bc_fwd(5, "Af5", 0, 14, 0, bd_rep(5))
        b.bc_fwd(5, "Af5", 14, 14, 1, bd_rep(5))

        # ============ inverse ============
        b.inv_a(5, bsrc_rep(5, None))
        b.inv_bc(5, lambda c: b.dram[f"VA4rec{c}"])
        b.inv_a(4, bsrc_rep(4, "VA4rec"))
        b.inv_bc(4, lambda c: b.dram[f"VA3rec{c}"])
        b.inv_a(3, bsrc_rep(3, "VA3rec"))
        b.inv_bc(3, lambda c: b.dram[f"VA2rec{c}"])
        b.inv_a(2, bsrc2, quads=[(0, 0)])
        b.inv_bc(2, lambda c: b.dram[f"VA1rec{c}"])
        b.inv_a(1, bsrc1, quads=[(0, 0)])
        b.inv_bc(1, lambda c: outs[c])

    nc.compile()
    return nc


_CACHE = {}


def make_in_maps(x_real, x_imag):
    x_real = np.ascontiguousarray(x_real, dtype=np.float32)
    x_imag = np.ascontiguousarray(x_imag, dtype=np.float32)
    in_maps = []
    for c in range(NCORE):
        m = host_matrices(c)
        slab_lo = 32 * c - 6
        im = {}
        for comp, x in (("re", x_real), ("im", x_imag)):
            s = np.zeros((44, 256, 256), dtype=NPB)
            g0, g1 = max(0, slab_lo), min(256, slab_lo + 44)
            s[g0 - slab_lo:g1 - slab_lo] = x[g0:g1]
            im[f"xs_{comp}"] = s
        for k, v in m.items():
            im[k] = v.astype(NPB)
        in_maps.append(im)
    return in_maps


def kernel(x_real, x_imag, alpha):
    thresh = 1e-3 * float(np.asarray(alpha))
    if thresh not in _CACHE:
        _CACHE[thresh] = build_program(thresh)
    nc = _CACHE[thresh]

    in_maps = make_in_maps(x_real, x_imag)
    res = run_bass_kernel_spmd(nc, in_maps, core_ids=list(range(NCORE)))
    out = np.empty((256, 256, 256), dtype=np.complex64)
    for c in range(NCORE):
        r = res.results[c]
        out[32 * c:32 * c + 32] = r["out_re"] + 1j * r["out_im"]
    return out


# revision 44
# speedup vs baseline: 1.0706x; 1.0138x over previous
"""Trainium2 Bass kernel for nn_L1Wav: 5-level 3D db4 wavelet soft-threshold
denoising of a 256^3 complex volume, SPMD over 8 NeuronCores.

Math notes (verified against the jax reference):
  - The deterministic rng(1000) shift is 0 and the unit-modulus phase cancels
    through the prox, so the computation is exactly:
    5-level 3D DWT -> complex soft-threshold -> inverse DWT.
  - Sharding: volume split along axis 0 (32 planes/core). Levels 1-2 are
    distributed; levels 3-5 replicated. One AllGather of the L1 approx band.

Implementation notes (v3):
  - All separable 2D (q,r) transform passes use a "data-stationary" two-step
    matmul form: step1 out1(r,q') = X(q,r)^T @ W(q,q'), step2
    out2(q',r') = out1(r,q')^T @ W(r,r').  No PE transposes.
  - bf16 matmul datapath (fp32 matmul costs 2 PE instructions per call;
    bf16 costs 1).  PSUM accumulation stays fp32; the soft-threshold factor
    f = 1 - t/|w| is computed in fp32.  End-to-end rel err ~1e-3 vs the
    2e-2 budget.
  - step1/step2 software-pipelined one group deep so the PE's LDWEIGHTS
    (whose lhsT is fresh data, not reusable weights) can hoist.
  - DMAs batched (multi-row stages) and rotated across SP/ACT/POOL queues
    (~0.6us issue cost each).
  - Detail bands stored AD-concatenated per (X,Y) so the inverse a-pass
    loads one contiguous (2h, L*L) tensor per quadrant.
"""
import sys
from contextlib import ExitStack

import numpy as np
import ml_dtypes

sys.path.insert(0, "/opt/trn_rl_repo")

import concourse.bass as bass
import concourse.mybir as mybir
import concourse.tile as tile
from concourse import bacc
from concourse.bass_utils import run_bass_kernel_spmd

DT = mybir.dt.float32
DB = mybir.dt.bfloat16
NPB = ml_dtypes.bfloat16
F = 8
DEC_LO = np.array([-0.010597401784997278, 0.032883011666982945, 0.030841381835986965,
                   -0.18703481171888114, -0.02798376941698385, 0.6308807679295904,
                   0.7148465705525415, 0.23037781330885523])
REC_LO = DEC_LO[::-1].copy()
REC_HI = np.array([((-1) ** n) * DEC_LO[n] for n in range(F)])
DEC_HI = REC_HI[::-1].copy()

NS = [256, 131, 69, 38, 22, 14]     # sizes level 0..5
NCORE = 8
COMPS = ("re", "im")
HALF = {1: 19, 2: 13, 3: 38, 4: 22, 5: 14}      # band rows per a-filter half
INV_OUT_ROWS = {1: 32, 2: 19, 3: 69, 4: 38, 5: 22}
EPS = 1e-30
# forward bc row-group (gb*2L <= 512) and DMA row batch (gb*gbuf rows)
BC_GB = {1: 1, 2: 3, 3: 6, 4: 11, 5: 14}
BC_GBUF = {1: 4, 2: 2, 3: 1, 4: 1, 5: 1}
# inverse bc row-group (gb*P <= 512) and stage rows
IBC_GB = {1: 2, 2: 3, 3: 7, 4: 13, 5: 18}
IBC_RSTAGE = {1: 8, 2: 10, 3: 23, 4: 38, 5: 22}


def W_mat(N, flt):
    L = (N + F - 1) // 2
    W = np.zeros((L, N), dtype=np.float32)
    for l in range(L):
        for j in range(F):
            n = 2 * l + 1 - j
            if 0 <= n < N:
                W[l, n] = flt[j]
    return W


def G_mat(L, crop, flt):
    G = np.zeros((crop, L), dtype=np.float32)
    for t in range(crop):
        for m in range(L):
            j = t + 6 - 2 * m
            if 0 <= j < F:
                G[t, m] = flt[j]
    return G


def host_matrices(core):
    """All weight matrices for one core (lhsT layout: (K, M))."""
    c = core
    m = {}
    for l in range(5):
        W2 = np.concatenate([W_mat(NS[l], DEC_LO), W_mat(NS[l], DEC_HI)], 0)
        m[f"WT{l + 1}"] = np.ascontiguousarray(W2.T)     # (N_{l-1}, 2*N_l)
        glo = G_mat(NS[l + 1], NS[l], REC_LO)
        ghi = G_mat(NS[l + 1], NS[l], REC_HI)
        m[f"IAB{l + 1}"] = np.ascontiguousarray(
            np.concatenate([glo.T, ghi.T], 0))           # (2*N_l, N_{l-1})
    # L1 fwd a-pass (per-core): A1 (38, 44) -> lhsT (44, 38)
    A1 = np.zeros((38, 44), dtype=np.float32)
    slab_lo = 32 * c - 6
    for half, flt in ((0, DEC_LO), (1, DEC_HI)):
        for i in range(19):
            l = 16 * c + i
            for k in range(44):
                n = slab_lo + k
                j = 2 * l + 1 - n
                if 0 <= j < F and 0 <= n < 256:
                    A1[half * 19 + i, k] = flt[j]
    m["A1T"] = np.ascontiguousarray(A1.T)
    # L2 fwd a-pass fused: per-core details (26 rows) + replicated full-lo (69)
    A2 = np.concatenate([W_mat(131, DEC_LO)[8 * c:8 * c + 13],
                         W_mat(131, DEC_HI)[8 * c:8 * c + 13]], 0)   # (26,131)
    A2F = np.concatenate([A2, W_mat(131, DEC_LO)], 0)                # (95,131)
    m["A2FT"] = np.ascontiguousarray(A2F.T)                          # (131,95)
    # L1 inv a-pass: core-independent (38, 32)
    G1a = np.zeros((32, 19), dtype=np.float32)
    G1d = np.zeros((32, 19), dtype=np.float32)
    for u in range(32):
        for v in range(19):
            j = u + 6 - 2 * v
            if 0 <= j < F:
                G1a[u, v] = REC_LO[j]
                G1d[u, v] = REC_HI[j]
    m["IA1"] = np.ascontiguousarray(np.concatenate([G1a.T, G1d.T], 0))
    # L2 inv a-pass (per-core)
    glo1 = G_mat(69, 131, REC_LO)
    ghi1 = G_mat(69, 131, REC_HI)
    g2a_full = glo1[16 * c:16 * c + 19, :]                    # (19, 69)
    g2a13 = glo1[16 * c:16 * c + 19, 8 * c:8 * c + 13]
    g2d13 = ghi1[16 * c:16 * c + 19, 8 * c:8 * c + 13]
    m["IA2"] = np.ascontiguousarray(np.concatenate([g2a13.T, g2d13.T], 0))
    m["IA2LL"] = np.ascontiguousarray(np.concatenate([g2a_full.T, g2d13.T], 0))
    return {k: v.astype(np.float32) for k, v in m.items()}


MAT_SHAPES = {k: v.shape for k, v in host_matrices(0).items()}


def chunks_of(total, size=128):
    return [(i, min(size, total - i)) for i in range(0, total, size)]


class Builder:
    def __init__(self, nc, tc, ctx, thresh):
        self.nc = nc
        self.tc = tc
        self.thresh = float(thresh)
        self.p_dram = ctx.enter_context(
            tc.tile_pool(name="dram", bufs=1, space=bass.MemorySpace.DRAM))
        self.p_wts = ctx.enter_context(tc.tile_pool(name="wts", bufs=1))
        self.p_work = ctx.enter_context(tc.tile_pool(name="work", bufs=1))
        self.p_psum = ctx.enter_context(
            tc.tile_pool(name="psum", bufs=1, space=bass.MemorySpace.PSUM))
        self.mats = {}
        self.dram = {}
        self.uid = 0
        self.dma_ld = 0
        self.dma_st = 0
        self.cp_rr = 0
        self.p1_rr = 0
        self.p2_rr = 0
        self.s1_rr = 0
        self.pool_ok = True

    def _id(self):
        self.uid += 1
        return self.uid

    # ---- helpers -----------------------------------------------------
    def dram_tile(self, name, shape, dtype=DB, addr_space="Local"):
        t = self.p_dram.tile(list(shape), dtype, name=name, tag=name,
                             addr_space=addr_space)
        self.dram[name] = t
        return t

    def sbuf(self, shape, tag, bufs=1, dtype=DB):
        return self.p_work.tile(list(shape), dtype, name=f"t{self._id()}",
                                tag=tag, bufs=bufs)

    def psum(self, shape, tag):
        return self.p_psum.tile(list(shape), DT, name=f"p{self._id()}",
                                tag=tag, bufs=1)

    def load(self, dst, src):
        """Input DMA, rotated across POOL/SP issue queues (ACT is busy with
        copies + threshold).  POOL is skipped while self.pool_ok is False
        (an in-flight collective on the Pool queue would head-of-line block
        loads emitted after it)."""
        self.dma_ld += 1
        if self.pool_ok:
            if self.dma_ld % 2 == 0:
                self.nc.sync.dma_start(dst, src)
            else:
                self.nc.gpsimd.dma_start(dst, src)
        else:
            if self.dma_ld % 2 == 0:
                self.nc.sync.dma_start(dst, src)
            else:
                self.nc.scalar.dma_start(dst, src)

    def store(self, dst, src):
        """Output DMA, rotated across ACT/SP/POOL queues."""
        self.dma_st += 1
        e = self.dma_st % 3
        if e == 0 or not self.pool_ok:
            self.nc.sync.dma_start(dst, src)
        elif e == 1:
            self.nc.scalar.dma_start(dst, src)
        else:
            self.nc.gpsimd.dma_start(dst, src)

    def copy(self, dst, src):
        """PSUM->SBUF copy (casts), alternating ACT/DVE."""
        self.cp_rr += 1
        if self.cp_rr % 2 == 0:
            self.nc.scalar.copy(dst, src)
        else:
            self.nc.vector.tensor_copy(dst, src)

    def load_mat(self, name, dram_ap):
        K, M = dram_ap.shape
        tiles = []
        for (k0, kn) in chunks_of(K):
            t = self.p_wts.tile([kn, M], DB, name=f"{name}_{k0}",
                                tag=f"{name}_{k0}", bufs=1)
            self.nc.sync.dma_start(t[:, :], dram_ap[k0:k0 + kn, :])
            tiles.append((t, k0, kn))
        self.mats[name] = tiles

    def mat_rows(self, name, row0, rown):
        """Pieces of matrix rows [row0, row0+rown) split at SBUF chunk
        boundaries: list of (sbuf_ap_all_cols, local_offset, piece_rows)."""
        out = []
        for (t, k0, kn) in self.mats[name]:
            lo = max(row0, k0)
            hi = min(row0 + rown, k0 + kn)
            if lo < hi:
                out.append((t[lo - k0:hi - k0, :], lo - row0, hi - lo))
        assert sum(p[2] for p in out) == rown, f"{name} rows {row0}+{rown}"
        return out

    # ---- soft-threshold: th = w * (1 - t/|w|), f in fp32 -------------
    def soft_pair(self, s_re, s_im, th_re, th_im, full_shape, sb):
        nc = self.nc
        t = self.thresh
        ta = self.sbuf(full_shape, "TH_A", bufs=2, dtype=DT)[:, :sb, :]
        tb = self.sbuf(full_shape, "TH_B", bufs=2, dtype=DT)[:, :sb, :]
        tc_ = self.sbuf(full_shape, "TH_C", bufs=2, dtype=DT)[:, :sb, :]
        nc.scalar.square(ta, s_re)
        nc.scalar.square(tb, s_im)
        nc.vector.scalar_tensor_tensor(tc_, ta, EPS, tb,
                                       mybir.AluOpType.add,
                                       mybir.AluOpType.add)   # re^2+im^2+eps
        nc.vector.reciprocal_approx_fast(ta, tc_)             # 1/|w|^2
        nc.scalar.sqrt(tb, ta)                                # 1/|w|
        nc.vector.tensor_scalar(tc_, tb, -t, 1.0,
                                mybir.AluOpType.mult,
                                mybir.AluOpType.add)          # f = 1 - t/|w|
        nc.vector.tensor_mul(th_re, s_re, tc_)
        nc.vector.tensor_mul(th_im, s_im, tc_)

    # ---- a-pass (forward levels): dst = lhsT^T @ src ----------------
    def fwd_a(self, lname, src_of, dsts, n, stage=2048):
        """src_of(comp) -> flat (K, n*n) AP, or a list of
        (dst_view_fn, src_ap) loader pairs per lhsT K-chunk where
        dst_view_fn(it) shapes the SBUF destination and src_ap is sliced
        [..., s0:s0+sn] on its last axis.
        dsts: list of (dst_of, out_r0, rn, dst_r0)."""
        nc = self.nc
        lhsT = self.mats[lname]
        M = max(r0 + rn for (_, r0, rn, _) in dsts)
        tot = n * n
        pend = []               # deferred stores (one stage deep)
        for comp in COMPS:
            srcf = src_of(comp)
            multi = isinstance(srcf, list)
            for s0 in range(0, tot, stage):
                sn = min(stage, tot - s0)
                its = []
                for i, (lt, k0, kn) in enumerate(lhsT):
                    it = self.sbuf([kn, stage], f"FA_IN{i}", bufs=2)
                    if multi:
                        for (d0, dn, sap) in srcf[i]:
                            self.load(it[d0:d0 + dn, :sn],
                                      sap[:, s0:s0 + sn])
                    else:
                        self.load(it[:, :sn], srcf[k0:k0 + kn, s0:s0 + sn])
                    its.append(it)
                ot = self.sbuf([M, stage], "FA_OUT", bufs=2)
                for t0 in range(0, sn, 512):
                    tn = min(512, sn - t0)
                    p = self.psum([M, 512], f"PF{(t0 // 512) % 2}")
                    for i, (lt, k0, kn) in enumerate(lhsT):
                        nc.tensor.matmul(p[:, :tn], lt[:, :],
                                         its[i][:, t0:t0 + tn],
                                         start=(i == 0),
                                         stop=(i == len(lhsT) - 1))
                    self.copy(ot[:, t0:t0 + tn], p[:, :tn])
                for (dst_of, r0, rn, d0) in dsts:
                    self.store(dst_of(comp)[d0:d0 + rn, s0:s0 + sn],
                               ot[r0:r0 + rn, :sn])

    # ---- generic forward bc-pass (data-stationary, row-grouped) ------
    def bc_fwd(self, lvl, src_name, g_base, g_count, af, bd, out_cols=None):
        """2D transform of rows [g_base, g_base+g_count) of {src_name}{comp}.
        bd(comp, af, X, Y, g0loc, rows, rx0, h) -> (dest_ap, use_th) | None,
        dest_ap shaped (h, rows, <=L) [already rearranged l b n]."""
        nc = self.nc
        Q = NS[lvl - 1]
        L = NS[lvl]
        cols = out_cols if out_cols is not None else 2 * L
        WT = self.mats[f"WT{lvl}"]
        qch = chunks_of(Q)
        mch = chunks_of(cols)
        gb = BC_GB[lvl]
        G = gb * BC_GBUF[lvl]               # rows staged per DMA batch
        tail = None                          # deferred threshold+stores
        for s0 in range(0, g_count, G):
            sb = min(G, g_count - s0)
            s3 = {}
            pending = []                     # [(s1, s3g, b0, bn), ...]
            # all input loads first, so the previous stage's stores (which
            # wait on its threshold chain) don't head-of-line block them
            ins = {}
            for comp in COMPS:
                src = self.dram[f"{src_name}{comp}"]
                row0 = g_base + s0
                its = []
                for qi, (q0, qn) in enumerate(qch):
                    it = self.sbuf([qn, G, Q], f"IN{qi}", bufs=2)
                    self.load(it[:, :sb, :],
                              src[row0:row0 + sb, q0:q0 + qn, :]
                              .rearrange("b q n -> q b n"))
                    its.append(it)
                ins[comp] = its
            for comp in COMPS:
                its = ins[comp]
                s3g = [self.sbuf([mn, G, cols], f"S3_{mi}_{comp}", bufs=2)
                       for mi, (m0, mn) in enumerate(mch)]
                s3[comp] = s3g
                for b0 in range(0, sb, gb):
                    bn = min(gb, sb - b0)
                    # step1 for this group
                    self.s1_rr += 1
                    s1 = []
                    for ri, (r0, rn) in enumerate(qch):
                        self.p1_rr += 1
                        p1 = self.psum([rn, gb, cols], f"P1{self.p1_rr % 4}")
                        for b in range(bn):
                            for qi, (q0, qn) in enumerate(qch):
                                nc.tensor.matmul(
                                    p1[:, b, :],
                                    its[qi][:, b0 + b, r0:r0 + rn],
                                    WT[qi][0][:, :cols],
                                    start=(qi == 0),
                                    stop=(qi == len(qch) - 1))
                        s = self.sbuf([rn, gb, cols],
                                      f"S1_{ri}_{self.s1_rr % 3}", bufs=2)
                        self.copy(s[:, :bn, :], p1[:, :bn, :])
                        s1.append(s)
                    # delayed step2 (software pipeline, depth 2)
                    pending.append((s1, s3g, b0, bn))
                    if len(pending) > 2:
                        self._bc_step2(*pending.pop(0), gb, qch, mch, WT,
                                       cols)
                if comp == COMPS[-1]:
                    while pending:
                        self._bc_step2(*pending.pop(0), gb, qch, mch, WT,
                                       cols)
            self._bc_tail(s0, sb, s3, lvl, af, bd, mch, L, cols, G)

    def _bc_tail(self, s0, sb, s3, lvl, af, bd, mch, L, cols, G):
        """Threshold + band writes for one staged row batch."""
        for mi, (m0, mn) in enumerate(mch):
            pieces = []
            for X in (0, 1):
                lo = max(m0, X * L)
                hi = min(m0 + mn, (X + 1) * L)
                if lo >= hi:
                    continue
                rr0, h = lo - m0, hi - lo
                rx0 = lo - X * L
                for Y in (0, 1):
                    if Y * L >= cols:
                        continue
                    for comp in COMPS:
                        d = bd(comp, af, X, Y, s0, sb, rx0, h)
                        if d is not None:
                            pieces.append((comp, X, Y, rr0, h, d[0],
                                           d[1]))
            th = {}
            if any(p[6] for p in pieces):
                for comp in COMPS:
                    th[comp] = self.sbuf([mn, G, cols],
                                         f"THO_{mi}_{comp}", bufs=2)
                self.soft_pair(s3["re"][mi][:, :sb, :],
                               s3["im"][mi][:, :sb, :],
                               th["re"][:, :sb, :],
                               th["im"][:, :sb, :],
                               [mn, G, cols], sb)
            for (comp, X, Y, rr0, h, dest, use_th) in pieces:
                st = th[comp] if use_th else s3[comp][mi]
                c1 = min((Y + 1) * L, cols)
                self.store(dest, st[rr0:rr0 + h, :sb, Y * L:c1])

    def _bc_step2(self, s1, s3g, b0, bn, gb, qch, mch, WT, cols):
        nc = self.nc
        for mi, (m0, mn) in enumerate(mch):
            self.p2_rr += 1
            p2 = self.psum([mn, gb, cols], f"P2{self.p2_rr % 2}")
            for b in range(bn):
                for ri, (r0, rn) in enumerate(qch):
                    nc.tensor.matmul(
                        p2[:, b, :],
                        s1[ri][:, b, m0:m0 + mn],
                        WT[ri][0][:, :cols],
                        start=(ri == 0),
                        stop=(ri == len(qch) - 1))
            self.copy(s3g[mi][:, b0:b0 + bn, :], p2[:, :bn, :])

    # ---- inverse a-pass ----------------------------------------------
    def inv_a(self, lvl, band_src, stage=2048, quads=None):
        """band_src(comp, X, Y) -> ([(flat_ap, k0, kn)...], lname).
        Writes OC{lvl}{comp}{Y} flat at X*L*L column offset."""
        nc = self.nc
        L = NS[lvl]
        M = INV_OUT_ROWS[lvl]
        tot = L * L
        if quads is None:
            quads = [(X, Y) for X in (0, 1) for Y in (0, 1)]
        pend = []
        for comp in COMPS:
            for X, Y in quads:
                if True:
                    pieces, lname = band_src(comp, X, Y)
                    lt, _, K = self.mats[lname][0]
                    dst = self.dram[f"OC{lvl}{comp}{Y}"].rearrange(
                        "a l n -> a (l n)")
                    for s0 in range(0, tot, stage):
                        sn = min(stage, tot - s0)
                        it = self.sbuf([K, stage], "FA_IN0", bufs=2)
                        for (ap, k0, kn) in pieces:
                            self.load(it[k0:k0 + kn, :sn],
                                      ap[:, s0:s0 + sn])
                        ot = self.sbuf([M, stage], "FA_OUT", bufs=2)
                        for t0 in range(0, sn, 512):
                            tn = min(512, sn - t0)
                            p = self.psum([M, 512], f"PF{(t0 // 512) % 2}")
                            nc.tensor.matmul(p[:, :tn], lt[:, :],
                                             it[:, t0:t0 + tn],
                                             start=True, stop=True)
                            self.copy(ot[:, t0:t0 + tn], p[:, :tn])
                        self.store(
                            dst[:, X * tot + s0:X * tot + s0 + sn],
                            ot[:, :sn])

    # ---- inverse bc-pass (data-stationary, row-grouped) --------------
    def inv_bc(self, lvl, out_dest):
        """OC{lvl}{comp}{Y} (rows, 2L, L) -> out_dest(comp) (rows, P, P)."""
        nc = self.nc
        rows = INV_OUT_ROWS[lvl]
        L = NS[lvl]
        twoL = 2 * L
        P = NS[lvl - 1]
        iname = f"IAB{lvl}"
        IAB = self.mats[iname]
        lch = chunks_of(twoL)
        nch = chunks_of(L)
        pch = chunks_of(P)
        gb = IBC_GB[lvl]
        rstage = IBC_RSTAGE[lvl]
        sdt = DT if lvl == 1 else DB
        stag = "S2F" if lvl == 1 else "S2B"
        # step2 K pieces: (Y, ni, pk0, pkn, rhs_ap)
        klist = []
        for Y in (0, 1):
            for ni, (n0, nn) in enumerate(nch):
                for (rhs, pk0, pkn) in self.mat_rows(f"IABY{lvl}{Y}", n0, nn):
                    klist.append((Y, ni, pk0, pkn, rhs))
        for comp in COMPS:
            dst = out_dest(comp)
            pending = []
            for s0 in range(0, rows, rstage):
                sb = min(rstage, rows - s0)
                oc = {}
                for Y in (0, 1):
                    src = self.dram[f"OC{lvl}{comp}{Y}"]
                    for li, (l0, ln) in enumerate(lch):
                        t = self.sbuf([ln, rstage, L],
                                      f"IN{2 * li + Y}", bufs=2)
                        self.load(t[:, :sb, :],
                                  src[s0:s0 + sb, l0:l0 + ln, :]
                                  .rearrange("b l n -> l b n"))
                        oc[(Y, li)] = t
                for b0 in range(0, sb, gb):
                    bn = min(gb, sb - b0)
                    self.s1_rr += 1
                    sU = {}
                    for Y in (0, 1):
                        for ni, (n0, nn) in enumerate(nch):
                            self.p1_rr += 1
                            p1 = self.psum([nn, gb, P],
                                           f"P1{self.p1_rr % 4}")
                            for b in range(bn):
                                for li, (l0, ln) in enumerate(lch):
                                    nc.tensor.matmul(
                                        p1[:, b, :],
                                        oc[(Y, li)][:, b0 + b, n0:n0 + nn],
                                        IAB[li][0][:, :],
                                        start=(li == 0),
                                        stop=(li == len(lch) - 1))
                            s = self.sbuf([nn, gb, P],
                                          f"S1_{2 * Y + ni}_{self.s1_rr % 3}",
                                          bufs=2)
                            self.copy(s[:, :bn, :], p1[:, :bn, :])
                            sU[(Y, ni)] = s
                    pending.append((sU, s0 + b0, bn))
                    if len(pending) > 2:
                        self._ibc_step2(*pending.pop(0), gb, pch, klist, P,
                                        dst, sdt, stag)
            while pending:
                self._ibc_step2(*pending.pop(0), gb, pch, klist, P, dst,
                                sdt, stag)

    def _ibc_step2(self, sU, g0, bn, gb, pch, klist, P, dst, sdt, stag):
        nc = self.nc
        for mi, (m0, mn) in enumerate(pch):
            self.p2_rr += 1
            p2 = self.psum([mn, gb, P], f"P2{self.p2_rr % 2}")
            for b in range(bn):
                for k, (Y, ni, pk0, pkn, rhs) in enumerate(klist):
                    nc.tensor.matmul(
                        p2[:, b, :],
                        sU[(Y, ni)][pk0:pk0 + pkn, b, m0:m0 + mn],
                        rhs,
                        start=(k == 0),
                        stop=(k == len(klist) - 1))
            s2 = self.sbuf([mn, gb, P], stag, bufs=3, dtype=sdt)
            self.copy(s2[:, :bn, :], p2[:, :bn, :])
            self.store(dst[g0:g0 + bn, m0:m0 + mn, :]
                       .rearrange("b m n -> m b n"),
                       s2[:, :bn, :])


def build_program(thresh, use_collective=True):
    nc = bacc.Bacc("TRN2", target_bir_lowering=False, debug=False,
                   num_devices=NCORE)
    ext = {}
    for comp in COMPS:
        ext[f"xs_{comp}"] = nc.dram_tensor(f"xs_{comp}", [44, 256, 256], DB,
                                           kind="ExternalInput").ap()
    for name, shp in MAT_SHAPES.items():
        ext[name] = nc.dram_tensor(name, list(shp), DB,
                                   kind="ExternalInput").ap()
    outs = {}
    for comp in COMPS:
        outs[comp] = nc.dram_tensor(f"out_{comp}", [32, 256, 256], DT,
                                    kind="ExternalOutput").ap()

    with tile.TileContext(nc) as tc, ExitStack() as ctx:
        b = Builder(nc, tc, ctx, thresh)

        for name in MAT_SHAPES:
            b.load_mat(name, ext[name])
        for lvl in range(1, 6):
            Ll = NS[lvl]
            for Y in (0, 1):
                b.load_mat(f"IABY{lvl}{Y}",
                           ext[f"IAB{lvl}"][Y * Ll:(Y + 1) * Ll, :])

        for comp in COMPS:
            b.dram[f"xs{comp}"] = ext[f"xs_{comp}"]
            b.dram_tile(f"Af1{comp}", (38, 256, 256))
            b.dram_tile(f"Af2{comp}", (26, 131, 131))
            b.dram_tile(f"Af2F{comp}", (69, 131, 131))
            b.dram_tile(f"Af3{comp}", (76, 69, 69))
            b.dram_tile(f"Af4{comp}", (44, 38, 38))
            b.dram_tile(f"Af5{comp}", (28, 22, 22))
            b.dram_tile(f"VA1full{comp}", (131, 131, 131))
            b.dram_tile(f"VA2full{comp}", (69, 69, 69))
            for lvl, L in ((1, 131), (2, 69), (3, 38), (4, 22), (5, 14)):
                for X in (0, 1):
                    for Y in (0, 1):
                        b.dram_tile(f"B{lvl}{comp}{X}{Y}",
                                    (2 * HALF[lvl], L, L))
            b.dram_tile(f"VA1rec{comp}", (19, 131, 131))
            b.dram_tile(f"VA2rec{comp}", (69, 69, 69))
            b.dram_tile(f"VA3rec{comp}", (38, 38, 38))
            b.dram_tile(f"VA4rec{comp}", (22, 22, 22))
            for lvl, L in ((1, 131), (2, 69), (3, 38), (4, 22), (5, 14)):
                for Y in (0, 1):
                    b.dram_tile(f"OC{lvl}{comp}{Y}",
                                (INV_OUT_ROWS[lvl], 2 * L, L))
        ag1_in = b.dram_tile("ag1_in", (38, 131, 131))
        ag1_out = b.dram_tile("ag1_out", (NCORE * 38, 131, 131),
                              addr_space="Shared")

        def flat(name):
            return lambda comp: b.dram[f"{name}{comp}"].rearrange(
                "a b c -> a (b c)")

        # ============ forward ============
        b.fwd_a("A1T",
                lambda c: ext[f"xs_{c}"].rearrange("a b c -> a (b c)"),
                [(flat("Af1"), 0, 38, 0)], 256)

        def bd1(comp, af, X, Y, g0, rows, rx0, h):
            if af == 0 and X == 0 and Y == 0:
                ci = 0 if comp == "re" else 1
                return (ag1_in[ci * 19 + g0:ci * 19 + g0 + rows,
                               rx0:rx0 + h, :]
                        .rearrange("b l n -> l b n"), False)
            return (b.dram[f"B1{comp}{X}{Y}"]
                    [af * 19 + g0:af * 19 + g0 + rows, rx0:rx0 + h, :]
                    .rearrange("b l n -> l b n"), True)

        # inverse band sources (used both for early off-critical-path
        # quadrants and in the inverse chain)
        def bsrc_rep(lvl, va_rec):
            h0 = HALF[lvl]

            def f(comp, X, Y):
                bt = b.dram[f"B{lvl}{comp}{X}{Y}"].rearrange(
                    "a b c -> a (b c)")
                if X == 0 and Y == 0 and lvl != 5:
                    va = b.dram[f"{va_rec}{comp}"].rearrange(
                        "a b c -> a (b c)")
                    return ([(va, 0, h0), (bt[h0:2 * h0], h0, h0)],
                            f"IAB{lvl}")
                return ([(bt, 0, 2 * h0)], f"IAB{lvl}")
            return f

        def bsrc2(comp, X, Y):
            bt = b.dram[f"B2{comp}{X}{Y}"].rearrange("a b c -> a (b c)")
            if X == 0 and Y == 0:
                va = b.dram[f"VA2rec{comp}"].rearrange("a b c -> a (b c)")
                return ([(va, 0, 69), (bt[13:26], 69, 13)], "IA2LL")
            return ([(bt, 0, 26)], "IA2")

        def bsrc1(comp, X, Y):
            bt = b.dram[f"B1{comp}{X}{Y}"].rearrange("a b c -> a (b c)")
            if X == 0 and Y == 0:
                va = b.dram[f"VA1rec{comp}"].rearrange("a b c -> a (b c)")
                return ([(va, 0, 19), (bt[19:38], 19, 19)], "IA1")
            return ([(bt, 0, 38)], "IA1")

        NON00 = [(0, 1), (1, 0), (1, 1)]

        b.bc_fwd(1, "Af1", 0, 19, 0, bd1)

        if use_collective:
            nc.gpsimd.collective_compute(
                "AllGather", mybir.AluOpType.bypass,
                ins=[ag1_in.opt()], outs=[ag1_out.opt()],
                replica_groups=[list(range(NCORE))])
        else:
            nc.sync.dma_start(ag1_out[0:38], ag1_in[0:38])

        b.pool_ok = False
        b.bc_fwd(1, "Af1", 19, 19, 1, bd1)
        b.pool_ok = True

        # L2 a-pass reads the gathered L1 approx band directly from
        # ag1_out: VA1 row 16k+j lives at ag1_out row 38k + ci*19 + j.
        # Simple 2D row-slices only (dependency tracking vs the collective
        # write is reliable for these).
        agf = ag1_out.rearrange("a b c -> a (b c)")

        def a2f_src(comp):
            ci = 0 if comp == "re" else 1
            c0 = []
            for k in range(7):
                c0.append((16 * k, 16,
                           agf[38 * k + ci * 19:38 * k + ci * 19 + 16]))
            c0.append((112, 16,
                       agf[38 * 7 + ci * 19:38 * 7 + ci * 19 + 16]))
            c1 = [(0, 3, agf[38 * 7 + ci * 19 + 16:38 * 7 + ci * 19 + 19])]
            return [c0, c1]

        b.fwd_a("A2FT", a2f_src,
                [(flat("Af2"), 0, 26, 0), (flat("Af2F"), 26, 69, 0)], 131)

        # L2 lo-lo (replicated, unthresholded) -> VA2full
        def bd_ll(comp, af, X, Y, g0, rows, rx0, h):
            return (b.dram[f"VA2full{comp}"][g0:g0 + rows, rx0:rx0 + h, :]
                    .rearrange("b l n -> l b n"), False)

        b.bc_fwd(2, "Af2F", 0, 69, 0, bd_ll, out_cols=69)

        def bd_rep(lvl):
            def f(comp, af, X, Y, g0, rows, rx0, h):
                h0 = HALF[lvl]
                if af == 0 and X == 0 and Y == 0:
                    if lvl == 2:
                        return None       # aaa2 comes from the bc_ll pass
                    use_th = (lvl == 5)
                    return (b.dram[f"B{lvl}{comp}00"][g0:g0 + rows,
                                                      rx0:rx0 + h, :]
                            .rearrange("b l n -> l b n"), use_th)
                return (b.dram[f"B{lvl}{comp}{X}{Y}"]
                        [af * h0 + g0:af * h0 + g0 + rows, rx0:rx0 + h, :]
                        .rearrange("b l n -> l b n"), True)
            return f

        b.bc_fwd(2, "Af2", 0, 13, 0, bd_rep(2))
        b.bc_fwd(2, "Af2", 13, 13, 1, bd_rep(2))

        b.fwd_a("WT3", flat("VA2full"), [(flat("Af3"), 0, 76, 0)], 69)
        b.bc_fwd(3, "Af3", 0, 38, 0, bd_rep(3))
        b.bc_fwd(3, "Af3", 38, 38, 1, bd_rep(3))
        # off-critical-path: L2 inverse a-pass detail quadrants (only need
        # the forward B2 bands) — fills the small-level trough
        b.inv_a(2, bsrc2, quads=NON00)

        b.fwd_a("WT4",
                lambda c: b.dram[f"B3{c}00"].rearrange("a b c -> a (b c)")
                [0:38], [(flat("Af4"), 0, 44, 0)], 38)
        b.bc_fwd(4, "Af4", 0, 22, 0, bd_rep(4))
        b.bc_fwd(4, "Af4", 22, 22, 1, bd_rep(4))
        # off-critical-path: L1 inverse a-pass detail quadrants
        b.inv_a(1, bsrc1, quads=NON00)

        b.fwd_a("WT5",
                lambda c: b.dram[f"B4{c}00"].rearrange("a b c -> a (b c)")
                [0:22], [(flat("Af5"), 0, 28, 0)], 22)
        b.bc_fwd(5, "Af5", 0, 14, 0, bd_rep(5))
        b.bc_fwd(5, "Af5", 14, 14, 1, bd_rep(5))

        # ============ inverse ============
        b.inv_a(5, bsrc_rep(5, None))
        b.inv_bc(5, lambda c: b.dram[f"VA4rec{c}"])
        b.inv_a(4, bsrc_rep(4, "VA4rec"))
        b.inv_bc(4, lambda c: b.dram[f"VA3rec{c}"])
        b.inv_a(3, bsrc_rep(3, "VA3rec"))
        b.inv_bc(3, lambda c: b.dram[f"VA2rec{c}"])
        b.inv_a(2, bsrc2, quads=[(0, 0)])
        b.inv_bc(2, lambda c: b.dram[f"VA1rec{c}"])
        b.inv_a(1, bsrc1, quads=[(0, 0)])
        b.inv_bc(1, lambda c: outs[c])

    nc.compile()
    return nc


_CACHE = {}


def make_in_maps(x_real, x_imag):
    x_real = np.ascontiguousarray(x_real, dtype=np.float32)
    x_imag = np.ascontiguousarray(x_imag, dtype=np.float32)
    in_maps = []
    for c in range(NCORE):
        m = host_matrices(c)
        slab_lo = 32 * c - 6
        im = {}
        for comp, x in (("re", x_real), ("im", x_imag)):
            s = np.zeros((44, 256, 256), dtype=NPB)
            g0, g1 = max(0, slab_lo), min(256, slab_lo + 44)
            s[g0 - slab_lo:g1 - slab_lo] = x[g0:g1]
            im[f"xs_{comp}"] = s
        for k, v in m.items():
            im[k] = v.astype(NPB)
        in_maps.append(im)
    return in_maps


def kernel(x_real, x_imag, alpha):
    thresh = 1e-3 * float(np.asarray(alpha))
    if thresh not in _CACHE:
        _CACHE[thresh] = build_program(thresh)
    nc = _CACHE[thresh]

    in_maps = make_in_maps(x_real, x_imag)
    res = run_bass_kernel_spmd(nc, in_maps, core_ids=list(range(NCORE)))
    out = np.empty((256, 256, 256), dtype=np.complex64)
    for c in range(NCORE):
        r = res.results[c]
        out[32 * c:32 * c + 32] = r["out_re"] + 1j * r["out_im"]
    return out
